# revision 21
# baseline (speedup 1.0000x reference)
"""Distributed Trainium2 Bass kernel for a dense-transformer attention block.

Sharding (8 NeuronCores): core cid = 4*b + g
  - b = batch index (B=2), g = kv-head group (N_KV_HEADS=4)
  - Each core: its 4 query heads + 1 kv head (column-parallel wq/wk/wv),
    RoPE, causal GQA attention with pre-ictal bias, then ROW-parallel wo:
    LN2 stats via a tiny AllReduce of per-group partial moments, local
    partial out = xhat_g @ woT_g, summed+scattered across the group with
    per-2-tile ReduceScatters straight into the output tensor.
  - Host concatenates the 8 output shards (bf16 -> f32).

LN1 is folded into the matmuls: projections run on TRANSPOSED RAW x
(transposes start the moment x tiles land, independent of LN stats);
the per-row (mean, std) correction enters each projection PSUM as one
extra K=2 matmul with host-precomputed (-colsum(W), bias) rows, and the
1/std factor rides the RoPE cos/sin tables (cos_rs = cos * rs_bcast).
Per-row stats become rows via tiny PE transposes ([128,3] -> [3,128]
at column offsets of one PSUM tile). Compute dtype: bf16 matmul
operands, f32 PSUM, f32(r) stats rows in the aug/broadcast matmuls.
"""

import math
from contextlib import ExitStack

import numpy as np
import ml_dtypes

import concourse.bass as bass
import concourse.bacc as bacc
import concourse.mybir as mybir
import concourse.tile as tile
from concourse.bass_utils import run_bass_kernel_spmd

# Problem constants (hardcoded per spec nn_Attention_36120674959366)
B = 2
S = 2048
DIM = 2048
N_HEADS = 16
N_KV_HEADS = 4
HEAD_DIM = 128
NH_LOC = N_HEADS // N_KV_HEADS  # 4 q-heads per core
DQ_LOC = NH_LOC * HEAD_DIM      # 512
PRE_ICTAL_WINDOW = 10
PRE_ICTAL_BIAS = 2.0
LN_EPS = 1e-5
NEG_INF = -1e9

SQD = math.sqrt(HEAD_DIM)           # 11.3137085
INV_SQD = 1.0 / SQD
BIAS_SCALED = PRE_ICTAL_BIAS * SQD  # 22.627417
NEG_SCALED = NEG_INF * SQD          # -1.13137085e10

NT = S // 128                        # 16 tiles of 128 rows
NC = DIM // 128                      # 16 dim chunks

F32 = mybir.dt.float32
F32R = mybir.dt.float32r
BF16 = mybir.dt.bfloat16

# attention chunks of query tiles: (first_tile, n_tiles); last column is
# split 2+2 so the final ReduceScatter tail is short
CHUNKS = [(0, 4), (4, 4), (8, 4), (12, 2), (14, 2)]
GROUPS = [[0, 1, 2, 3], [4, 5, 6, 7]]

_CACHED = {}


def build_nc(debug=False):
    nc = bacc.Bacc("TRN2", target_bir_lowering=False, debug=False, num_devices=8)

    # ---- kernel I/O (per-core shards; same graph on all 8 cores) ----
    xs = nc.dram_tensor("xs", [S, DIM], BF16, kind="ExternalInput")
    wqT = nc.dram_tensor("wqT", [DIM, DQ_LOC], BF16, kind="ExternalInput")
    wkT = nc.dram_tensor("wkT", [DIM, HEAD_DIM], BF16, kind="ExternalInput")
    wvT = nc.dram_tensor("wvT", [DIM, HEAD_DIM], BF16, kind="ExternalInput")
    woT = nc.dram_tensor("woT", [DQ_LOC, DIM], BF16, kind="ExternalInput")
    augq = nc.dram_tensor("augq", [4, DQ_LOC], BF16, kind="ExternalInput")
    augk = nc.dram_tensor("augk", [4, HEAD_DIM], BF16, kind="ExternalInput")
    augv = nc.dram_tensor("augv", [4, HEAD_DIM], BF16, kind="ExternalInput")
    wob = nc.dram_tensor("wob", [128, DQ_LOC], BF16, kind="ExternalInput")
    cwos = nc.dram_tensor("cwos", [128, DQ_LOC], BF16, kind="ExternalInput")
    labels = nc.dram_tensor("labels", [S], BF16, kind="ExternalInput")
    cosT = nc.dram_tensor("cosT", [HEAD_DIM, S], BF16, kind="ExternalInput")
    sinT = nc.dram_tensor("sinT", [HEAD_DIM, S], BF16, kind="ExternalInput")
    ident = nc.dram_tensor("ident", [128, 128], BF16, kind="ExternalInput")
    identf = nc.dram_tensor("identf", [128, 128], F32, kind="ExternalInput")
    pswap = nc.dram_tensor("pswap", [128, 128], BF16, kind="ExternalInput")
    ctri = nc.dram_tensor("ctri", [128, 128], BF16, kind="ExternalInput")
    out = nc.dram_tensor("out", [S, DQ_LOC], BF16, kind="ExternalOutput")
    if debug:
        dbg_sthl = nc.dram_tensor("dbg_sthl", [128, 4, 6], F32, kind="ExternalOutput")
        dbg_strows = nc.dram_tensor("dbg_strows", [4, 512], BF16, kind="ExternalOutput")
        dbg_strowr = nc.dram_tensor("dbg_strowr", [2, 512], BF16, kind="ExternalOutput")
        dbg_cosrs = nc.dram_tensor("dbg_cosrs", [128, 512], BF16, kind="ExternalOutput")
        dbg_xrT = nc.dram_tensor("dbg_xrT", [128, NC, 512], BF16, kind="ExternalOutput")
        dbg_kT = nc.dram_tensor("dbg_kT", [128, S], BF16, kind="ExternalOutput")
        dbg_qT = nc.dram_tensor("dbg_qT", [128, S], BF16, kind="ExternalOutput")
        dbg_vaug = nc.dram_tensor("dbg_vaug", [128, NT, 132], BF16, kind="ExternalOutput")
        dbg_attn = nc.dram_tensor("dbg_attn", [128, 4, DQ_LOC], BF16, kind="ExternalOutput")
        dbg_mu2 = nc.dram_tensor("dbg_mu2", [128, 4], F32, kind="ExternalOutput")
        dbg_rs2 = nc.dram_tensor("dbg_rs2", [128, 4], F32, kind="ExternalOutput")
        dbg_osb = nc.dram_tensor("dbg_osb", [128, DIM], BF16, kind="ExternalOutput")
        dbg_kT2 = nc.dram_tensor("dbg_kT2", [128, S], BF16, kind="ExternalOutput")
        dbg_xrT2 = nc.dram_tensor("dbg_xrT2", [128, NC, 512], BF16, kind="ExternalOutput")
        dbg_strows2 = nc.dram_tensor("dbg_strows2", [4, 512], BF16, kind="ExternalOutput")
        dbg_cosrs2 = nc.dram_tensor("dbg_cosrs2", [128, 512], BF16, kind="ExternalOutput")
        dbg_raw2 = nc.dram_tensor("dbg_raw2", [128, 512], BF16, kind="ExternalOutput")
        dbg_xrT1 = nc.dram_tensor("dbg_xrT1", [128, NC, 512], BF16, kind="ExternalOutput")
        dbg_cosrs1 = nc.dram_tensor("dbg_cosrs1", [128, 512], BF16, kind="ExternalOutput")
        dbg_strows1 = nc.dram_tensor("dbg_strows1", [4, 512], BF16, kind="ExternalOutput")

    AF = mybir.ActivationFunctionType
    OP = mybir.AluOpType

    with tile.TileContext(nc) as tc, ExitStack() as st:
        pc = st.enter_context(tc.tile_pool(name="const", bufs=1))
        dr = st.enter_context(tc.tile_pool(name="dr", bufs=1, space="DRAM"))
        pw = st.enter_context(tc.tile_pool(name="wts", bufs=1))
        pm = st.enter_context(tc.tile_pool(name="work", bufs=1))
        ps = st.enter_context(tc.tile_pool(name="ps", bufs=1, space="PSUM"))

        # ======== x tile loads: one DMA per tile, sync/scalar alternate ===
        xt_tiles = {}

        def load_xt(i, eng=None):
            xt = pm.tile([128, DIM], BF16, tag="xt", bufs=8, name=f"xt_{i}")
            (eng or nc.sync).dma_start(out=xt[:, :],
                                       in_=xs[128 * i:128 * i + 128, :])
            xt_tiles[i] = xt

        for i in range(8):
            load_xt(i)

        # ======== constants / weights ========
        ident_sb = pc.tile([128, 128], BF16, tag="ident")
        identf_sb = pc.tile([128, 128], F32, tag="identf")
        pswap_sb = pc.tile([128, 128], BF16, tag="pswap")
        ctri_sb = pc.tile([128, 128], BF16, tag="ctri")
        cos_sb = pc.tile([128, S], BF16, tag="cos")
        sin_sb = pc.tile([128, S], BF16, tag="sin")
        augq_sb = pc.tile([4, DQ_LOC], BF16, tag="augq")
        augk_sb = pc.tile([4, HEAD_DIM], BF16, tag="augk")
        augv_sb = pc.tile([4, HEAD_DIM], BF16, tag="augv")
        wob_sb = pc.tile([128, DQ_LOC], BF16, tag="wob")
        cwos_sb = pc.tile([128, DQ_LOC], BF16, tag="cwos")
        wq_sb = pw.tile([128, NC, DQ_LOC], BF16, tag="wq")
        wk_sb = pw.tile([128, NC, HEAD_DIM], BF16, tag="wk")
        wv_sb = pw.tile([128, NC, HEAD_DIM], BF16, tag="wv")
        wo_sb = pw.tile([128, 4, DIM], BF16, tag="wo")

        pcs = tc.alloc_tile_pool(name="csum", bufs=1)
        lab_sb = pcs.tile([1, S], BF16, tag="lab")

        # scalar queue: small early stuff first, then big weights
        nc.scalar.dma_start(out=lab_sb[:, :],
                            in_=labels.ap().rearrange("(o s) -> o s", o=1))
        nc.scalar.dma_start(
            out=wk_sb[:, :, :],
            in_=wkT.ap().rearrange("(c p) n -> p c n", p=128))
        nc.scalar.dma_start(
            out=wv_sb[:, :, :],
            in_=wvT.ap().rearrange("(c p) n -> p c n", p=128))
        nc.scalar.dma_start(out=pswap_sb[:, :], in_=pswap[:, :])
        nc.scalar.dma_start(out=cos_sb[:, :], in_=cosT[:, :])
        nc.scalar.dma_start(out=sin_sb[:, :], in_=sinT[:, :])
        nc.scalar.dma_start(out=augk_sb[:, :], in_=augk[:, :])
        nc.scalar.dma_start(out=augv_sb[:, :], in_=augv[:, :])
        nc.scalar.dma_start(out=augq_sb[:, :], in_=augq[:, :])
        nc.scalar.dma_start(out=identf_sb[:, :], in_=identf[:, :])
        for q in range(2):
            nc.gpsimd.dma_start(
                out=wq_sb[:, 8 * q:8 * q + 8, :],
                in_=wqT.ap().rearrange("(c p) n -> p c n", p=128)
                    [:, 8 * q:8 * q + 8, :])
        nc.sync.dma_start(out=ctri_sb[:, :], in_=ctri[:, :])
        nc.sync.dma_start(out=ident_sb[:, :], in_=ident[:, :])

        def emit_late_weights():
            for q in range(2):
                nc.gpsimd.dma_start(
                    out=wo_sb[:, 2 * q:2 * q + 2, :],
                    in_=woT.ap().rearrange("(c p) n -> p c n", p=128)
                        [:, 2 * q:2 * q + 2, :])
            nc.gpsimd.dma_start(out=wob_sb[:, :], in_=wob[:, :])
            nc.gpsimd.dma_start(out=cwos_sb[:, :], in_=cwos[:, :])

        eps_sb = pc.tile([128, 1], F32, tag="eps")
        nc.vector.memset(eps_sb[:, :], LN_EPS)
        ones_col = pc.tile([1, 128], BF16, tag="ones_col")
        nc.vector.memset(ones_col[:, :], 1.0)
        ones2_bf = pc.tile([2, 128], BF16, tag="ones2_bf")
        nc.vector.memset(ones2_bf[:, :], 1.0)


        # ======== seizure-label cumulative sums -> colv/rb ========
        colv_bf = pc.tile([128, NT], BF16, tag="colv_bf")
        colv = pc.tile([128, NT], F32, tag="colv")
        rb_sb = pc.tile([128, S], BF16, tag="rb_sb")

        def emit_csum():
            # cumulative seizure counts are small ints -> bf16 exact
            csrow_bf = pcs.tile([1, S + 12], BF16, tag="csrow_bf")
            nc.gpsimd.memset(csrow_bf[:, 0:1], 0.0)
            nc.vector.tensor_tensor_scan(
                out=csrow_bf[:, 1:S + 1],
                data0=lab_sb[:, :],
                data1=lab_sb[:, :],
                initial=0.0,
                op0=OP.add,
                op1=OP.bypass,
            )
            for j in range(11):
                nc.gpsimd.tensor_copy(csrow_bf[:, S + 1 + j:S + 2 + j],
                                      csrow_bf[:, S:S + 1])

            # colv[p, kt] = cs[min(128*kt + p + 10, S-1)] via DRAM bounce +
            # XBAR transpose (bf16 exact for small counts)
            csbuf = dr.tile([S + 12], BF16, tag="csbuf")
            nc.sync.dma_start(out=csbuf.rearrange("(o s) -> o s", o=1),
                              in_=csrow_bf[:, :])
            cs16 = pcs.tile([NT, 128], BF16, tag="cs16")
            nc.sync.dma_start(
                out=cs16[:, :],
                in_=csbuf[11:11 + S].rearrange("(t p) -> t p", p=128),
            )
            nc.sync.dma_start_transpose(colv_bf[:, :], cs16[:, :])
            nc.gpsimd.tensor_copy(colv[:, :], colv_bf[:, :])

            # rb_sb[p, q] = csrow[q] broadcast down partitions (cnt_a source)
            for sgm in range(4):
                rbp = ps.tile([128, 1024], F32, tag="mega", bufs=2,
                              name=f"rbp_{sgm}")
                nc.tensor.matmul(rbp[:, 0:512], lhsT=ones_col[:, :],
                                 rhs=csrow_bf[:, 512 * sgm:512 * sgm + 512],
                                 start=True, stop=True)
                nc.vector.tensor_copy(rb_sb[:, 512 * sgm:512 * sgm + 512],
                                      rbp[:, 0:512])
            pcs.release()

        # ======== persistent QKV tiles ========
        qT = pm.tile([128, NH_LOC, S], BF16, tag="qT")
        kT = pm.tile([128, S], BF16, tag="kT")
        v_aug = pm.tile([128, NT, 132], BF16, tag="v_aug")
        nc.gpsimd.memset(v_aug[:, :, 128:129], 1.0)

        # ---------------- per-block raw transpose + LN1 stats ----------
        xrT_tiles = {}
        stats_tiles = {}
        strow_tiles = {}

        def emit_block_trans_stats(g4):
            # transposes of RAW x tiles (no stats dependency) + stats chain
            xrT = pm.tile([128, NC, 512], BF16, tag="xrT", bufs=2,
                          name=f"xrT_{g4}")
            tq = [nc.sync, nc.sync, nc.sync, nc.sync]
            for j4 in range(4):
                i = 4 * g4 + j4
                xt = xt_tiles[i]
                tq[j4].dma_start_transpose(
                    xrT[:, :, 128 * j4:128 * j4 + 128], xt[:, :])
            xrT_tiles[g4] = xrT

            # stats: per tile (mu, s, rs) plus bf16 hi/lo splits so the
            # aug matmuls can run in plain bf16 without precision loss.
            # stats3: [128,4,3] f32 (mu, s, rs) per-partition values
            # sthl:   [128,4,6] f32 cols (mu_hi, s_hi, mu_lo, s_lo, rs_hi,
            #         rs_lo), every value exactly bf16-representable after
            #         the post-transpose bf16 cast (lo carries the residual)
            stats3 = pm.tile([128, 4, 3], F32, tag="stats3", bufs=2,
                             name=f"st3_{g4}")
            sthl = pm.tile([128, 4, 6], F32, tag="sthl", bufs=2,
                           name=f"sthl_{g4}")
            hibf = pm.tile([128, 4, 3], BF16, tag="hibf", bufs=2,
                           name=f"hibf_{g4}")
            mv = pm.tile([128, 4, 2], F32, tag="mv", bufs=2, name=f"mv_{g4}")
            for j4 in range(4):
                i = 4 * g4 + j4
                xt = xt_tiles.pop(i)
                st6 = pm.tile([128, 4, 6], F32, tag="st6a", bufs=2)
                for a4 in range(4):
                    nc.vector.bn_stats(st6[:, a4, :],
                                       xt[:, 512 * a4:512 * a4 + 512])
                nc.vector.bn_aggr(mv[:, j4, :], st6[:, :, :])
            nc.vector.tensor_copy(stats3[:, :, 0], mv[:, :, 0])
            nc.scalar.activation(stats3[:, :, 1], mv[:, :, 1], AF.Sqrt,
                                 bias=eps_sb[:, :])
            nc.vector.reciprocal(stats3[:, :, 2], stats3[:, :, 1])
            stats_tiles[g4] = stats3
            # hi = f32(bf16(v)); lo = v - hi   (col pairs: 2k hi, 2k+1 ...)
            for k3, src_c in ((0, 0), (1, 1), (4, 2)):
                nc.vector.tensor_copy(hibf[:, :, src_c], stats3[:, :, src_c])
                nc.vector.tensor_copy(sthl[:, :, k3], hibf[:, :, src_c])
                nc.vector.tensor_sub(sthl[:, :, k3 + 2 - (k3 // 4) * 1],
                                     stats3[:, :, src_c], sthl[:, :, k3])
            stats_tiles[g4] = stats3

            sthl_tiles[g4] = sthl
            if debug and g4 == 0:
                nc.sync.dma_start(out=dbg_sthl[:, :, :], in_=sthl[:, :, :])
                nc.sync.dma_start(out=dbg_strows[:, :], in_=strows[:, :])
                nc.sync.dma_start(out=dbg_strowr[:, :], in_=strowr[:, :])
            if debug and g4 == 1:
                nc.sync.dma_start(out=dbg_strows1[:, :], in_=strows[:, :])
            if debug and g4 == 2:
                nc.sync.dma_start(out=dbg_strows2[:, :], in_=strows[:, :])

        # ---------------- stat rows (stats-gated; emitted inside qkv) ----
        sthl_tiles = {}

        def emit_rows(g4):
            # PE-transpose (f32) [128,4]->[4,128] (mu_hi,s_hi,mu_lo,s_lo)
            # and [128,2]->[2,128] (rs_hi,rs_lo) at col offsets of one PSUM
            # tile, then cast to bf16 rows (lossless: values are hi/lo)
            sthl = sthl_tiles.pop(g4)
            ptt = ps.tile([128, 1024], F32, tag="mega", bufs=2,
                          name=f"ptt_{g4}")
            for t in range(4):
                nc.tensor.matmul(ptt[0:4, 128 * t:128 * t + 128],
                                 lhsT=sthl[:, t, 0:4], rhs=identf_sb[:, :],
                                 is_transpose=True, skip_group_check=True)
                nc.tensor.matmul(ptt[0:2, 512 + 128 * t:512 + 128 * t + 128],
                                 lhsT=sthl[:, t, 4:6], rhs=identf_sb[:, :],
                                 is_transpose=True, skip_group_check=True)
            strows = pm.tile([4, 512], BF16, tag="strows", bufs=2,
                             name=f"strows_{g4}")
            nc.vector.tensor_copy(strows[:, :], ptt[0:4, 0:512])
            strowr = pm.tile([2, 512], BF16, tag="strowr", bufs=2,
                             name=f"strowr_{g4}")
            nc.vector.tensor_copy(strowr[:, :], ptt[0:2, 512:1024])
            strow_tiles[g4] = (strows, strowr)

        # ---------------- QKV projections ----------------
        cs_tiles = {}

        def emit_qkv(g4):
            sg = g4
            xrT = xrT_tiles[g4]
            stats3 = stats_tiles[g4]

            # stats-independent work first: V c-chunks (pv_acc slots) and
            # K c-chunks (open mega group); the stats-gated transposes, aug
            # matmuls and rope follow
            vps_tiles = []
            for j4 in range(4):
                i = 4 * g4 + j4
                vps = ps.tile([128, 132], F32, tag="pv_acc", bufs=4,
                              name=f"vps_{i}")
                reg = vps[:, 0:128]
                for c in range(NC):
                    nc.tensor.matmul(
                        reg, lhsT=xrT[:, c, 128 * j4:128 * j4 + 128],
                        rhs=wv_sb[:, c, :],
                        start=(c == 0), stop=False,
                        skip_group_check=True)
                vps_tiles.append(vps)
            pqk = ps.tile([128, 1024], F32, tag="mega", bufs=2,
                          name=f"pqk_{sg}")
            for c in range(NC):
                nc.tensor.matmul(pqk[:, 0:512], lhsT=wk_sb[:, c, :],
                                 rhs=xrT[:, c, 0:512],
                                 start=(c == 0), stop=False,
                                 skip_group_check=True)
            emit_rows(g4)
            strows, strowr = strow_tiles[g4]
            aug_mm = strows[:, :]   # rows: mu_hi, s_hi, mu_lo, s_lo
            rs_row = strowr[:, :]   # rows: rs_hi, rs_lo
            for j4 in range(4):
                i = 4 * g4 + j4
                reg = vps_tiles[j4][:, 0:128]
                nc.tensor.matmul(
                    reg,
                    lhsT=aug_mm[:, 128 * j4:128 * j4 + 128],
                    rhs=augv_sb[:, :],
                    start=False, stop=True,
                    skip_group_check=True)
                nc.vector.tensor_single_scalar(
                    out=v_aug[:, i, 0:128], in_=reg,
                    scalar=stats3[:, j4, 2:3], op=OP.mult)

            # rs broadcast -> cos_rs / sin_rs for this 512-col block
            rsb = ps.tile([128, 1024], F32, tag="mega", bufs=2,
                          name=f"rsb_{g4}")
            nc.tensor.matmul(rsb[:, 0:512], lhsT=ones2_bf[:, :],
                             rhs=rs_row, start=True, stop=True)
            cos_rs = pm.tile([128, 512], BF16, tag="cos_rs", bufs=2,
                             name=f"cos_rs_{g4}")
            sin_rs = pm.tile([128, 512], BF16, tag="sin_rs", bufs=2,
                             name=f"sin_rs_{g4}")
            nc.vector.tensor_mul(cos_rs[:, :], rsb[:, 0:512],
                                 cos_sb[:, 512 * sg:512 * sg + 512])
            nc.vector.tensor_mul(sin_rs[:, :], rsb[:, 0:512],
                                 sin_sb[:, 512 * sg:512 * sg + 512])
            cs_tiles[g4] = (cos_rs, sin_rs)

            def proj_rope(dst_sg, w_sb, h, waug, pq=None):
                # 16 c-chunk matmuls + 1 aug matmul -> psum (LN1 folded)
                if pq is None:
                    pq = ps.tile([128, 1024], F32, tag="mega", bufs=2,
                                 name=f"pq_{h}_{sg}")
                    for c in range(NC):
                        lhsT = w_sb[:, c, :] if h is None \
                            else w_sb[:, c, 128 * h:128 * h + 128]
                        nc.tensor.matmul(pq[:, 0:512], lhsT=lhsT,
                                         rhs=xrT[:, c, 0:512],
                                         start=(c == 0), stop=False)
                nc.tensor.matmul(pq[:, 0:512], lhsT=waug, rhs=aug_mm,
                                 start=False, stop=True,
                                 skip_group_check=True)
                # unscaled copy for the rope swap matmul
                raw = pm.tile([128, 512], BF16, tag="rope_raw", bufs=2)
                nc.vector.tensor_copy(raw[:, :], pq[:, 0:512])
                if debug and g4 == 2 and h is None:
                    nc.sync.dma_start(out=dbg_raw2[:, :], in_=raw[:, :])
                pw2 = ps.tile([128, 1024], F32, tag="mega", bufs=2,
                              name=f"pw2_{h}_{sg}")
                nc.tensor.matmul(pw2[:, 0:512], lhsT=pswap_sb[:, :],
                                 rhs=raw[:, :], start=True, stop=True)
                t1 = pm.tile([128, 512], BF16, tag="rope_t1", bufs=2)
                nc.vector.tensor_mul(t1[:, :], raw[:, :], cos_rs[:, :])
                t2 = pm.tile([128, 512], BF16, tag="rope_t2", bufs=2)
                nc.vector.tensor_mul(t2[:, :], pw2[:, 0:512], sin_rs[:, :])
                nc.vector.tensor_add(dst_sg, t1[:, :], t2[:, :])

            # K first (attention needs all K blocks), then Q heads
            proj_rope(kT[:, 512 * sg:512 * sg + 512], wk_sb, None,
                      augk_sb[:, :], pq=pqk)
            for h in range(NH_LOC):
                proj_rope(qT[:, h, 512 * sg:512 * sg + 512], wq_sb, h,
                          augq_sb[:, 128 * h:128 * h + 128])
            if debug and g4 == 0:
                nc.sync.dma_start(out=dbg_cosrs[:, :], in_=cos_rs[:, :])
                nc.sync.dma_start(out=dbg_xrT[:, :, :], in_=xrT[:, :, :])
            if debug and g4 == 1:
                nc.sync.dma_start(out=dbg_cosrs1[:, :], in_=cos_rs[:, :])
                nc.sync.dma_start(out=dbg_xrT1[:, :, :], in_=xrT[:, :, :])
                nc.sync.dma_start(out=dbg_kT2[:, :], in_=kT[:, :])
            if debug and g4 == 2:
                nc.sync.dma_start(out=dbg_cosrs2[:, :], in_=cos_rs[:, :])
                nc.sync.dma_start(out=dbg_xrT2[:, :, :], in_=xrT[:, :, :])

        # ---------------- attention ----------------
        def build_mg(ci):
            s0, w = CHUNKS[ci]
            kts = list(range(max(0, s0 - 1), s0 + w))
            mg = pm.tile([128, 5, 512], BF16, tag="mg", bufs=1,
                         name=f"mg_{ci}")
            nc.vector.memset(mg[:, :, :], 0.0)
            mg_idx = {}
            for slot, kt in enumerate(kts):
                mg_idx[kt] = slot
                if s0 <= kt < s0 + w:  # diag: qtile t == kt
                    j = kt - s0
                    sl = mg[:, slot, 128 * j:128 * j + 128]
                    nc.vector.tensor_scalar(
                        out=sl, in0=rb_sb[:, 128 * kt:128 * kt + 128],
                        scalar1=colv[:, kt:kt + 1], scalar2=BIAS_SCALED,
                        op0=OP.is_lt, op1=OP.mult,
                    )
                    nc.vector.tensor_add(sl, sl, ctri_sb[:, :])
                tprev = kt + 1
                if s0 <= tprev < s0 + w:  # prev: qtile t == kt + 1
                    j = tprev - s0
                    sl = mg[:, slot, 128 * j:128 * j + 128]
                    nc.vector.tensor_scalar(
                        out=sl, in0=rb_sb[:, 128 * tprev:128 * tprev + 128],
                        scalar1=colv[:, kt:kt + 1], scalar2=BIAS_SCALED,
                        op0=OP.is_lt, op1=OP.mult,
                    )
            return mg, mg_idx

        attn_tiles = {}

        def emit_attention(ci, mids=None, post_attn=None):
            s0, w = CHUNKS[ci]
            mg, mg_idx = build_mg(ci)
            attn_c = pm.tile([128, 4, DQ_LOC], BF16, tag="attn_c",
                             bufs=2, name=f"attn_c{ci}")
            attn_tiles[ci] = attn_c
            for h in range(NH_LOC):
                if mids is not None and h in mids:
                    mids[h]()
                nkt = s0 + w
                pvp = [ps.tile([128, 132], F32, tag="pv_acc", bufs=4,
                               name=f"pv_{h}_{ci}_{jj}")
                       for jj in range(w)]
                kt = 0
                while kt < nkt:
                    mega = ps.tile([128, 1024], F32, tag="mega", bufs=2,
                                   name=f"sc_{h}_{ci}_{kt}")
                    pair = [k2 for k2 in (kt, kt + 1) if k2 < nkt]
                    offs = []
                    for slot, k2 in enumerate(pair):
                        off = 128 * max(0, k2 - s0)
                        offs.append(off)
                        reg = mega[:, 512 * slot + off:
                                   512 * slot + 128 * w]
                        if k2 in mg_idx:
                            nc.tensor.matmul(
                                reg, lhsT=ident_sb[:, :],
                                rhs=mg[:, mg_idx[k2], off:128 * w],
                                start=True, stop=False)
                            nc.tensor.matmul(
                                reg,
                                lhsT=kT[:, 128 * k2:128 * k2 + 128],
                                rhs=qT[:, h, 128 * s0 + off:
                                       128 * (s0 + w)],
                                start=False, stop=True)
                        else:
                            nc.tensor.matmul(
                                reg,
                                lhsT=kT[:, 128 * k2:128 * k2 + 128],
                                rhs=qT[:, h, 128 * s0 + off:
                                       128 * (s0 + w)],
                                start=True, stop=True)
                    pt = pm.tile([128, 1024], BF16, tag="pt_sm", bufs=2)
                    for slot, k2 in enumerate(pair):
                        off = offs[slot]
                        nc.scalar.activation(
                            pt[:, 512 * slot + off:512 * slot + 128 * w],
                            mega[:, 512 * slot + off:512 * slot + 128 * w],
                            AF.Exp, scale=INV_SQD)
                    for slot, k2 in enumerate(pair):
                        for j in range(max(0, k2 - s0), w):
                            nc.tensor.matmul(
                                pvp[j][:, 0:129],
                                lhsT=pt[:, 512 * slot + 128 * j:
                                        512 * slot + 128 * j + 128],
                                rhs=v_aug[:, k2, 0:129],
                                start=(k2 == 0), stop=(k2 == s0 + j),
                                skip_group_check=True)
                    kt += len(pair)
                for j in range(w):
                    rcp = pm.tile([128, 1], F32, tag="rcp", bufs=4,
                                  name=f"rcp_{h}_{ci}_{j}")
                    nc.vector.reciprocal(rcp[:, :], pvp[j][:, 128:129])
                    nc.vector.tensor_single_scalar(
                        out=attn_c[:, j, 128 * h:128 * h + 128],
                        in_=pvp[j][:, 0:128],
                        scalar=rcp[:, :],
                        op=OP.mult)
            if post_attn is not None:
                post_attn()

        # ---------------- LN2 stats + AllReduce ----------------
        ln2_state = {}

        def emit_ln2_ar(ci):
            s0, w = CHUNKS[ci]
            attn_c = attn_tiles[ci]
            if debug and ci == 0:
                nc.sync.dma_start(out=dbg_attn[:, :, :], in_=attn_c[:, :, :])
            arloc = pm.tile([128, 4, 2], F32, tag="arloc", bufs=2,
                            name=f"arloc_{ci}")
            for j in range(w):
                st6 = pm.tile([128, 1, 6], F32, tag="st6b", bufs=2)
                nc.vector.bn_stats(st6[:, 0, :], attn_c[:, j, :])
                mvl = pm.tile([128, 2], F32, tag="mvl", bufs=4,
                              name=f"mvl_{ci}_{j}")
                nc.vector.bn_aggr(mvl[:, :], st6[:, :, :])
                # (mu_g, m2_g = var_g + mu_g^2)
                nc.vector.tensor_copy(arloc[:, j, 0:1], mvl[:, 0:1])
                sq = pm.tile([128, 1], F32, tag="sq", bufs=4)
                nc.vector.tensor_mul(sq[:, :], mvl[:, 0:1], mvl[:, 0:1])
                nc.vector.tensor_add(arloc[:, j, 1:2], mvl[:, 1:2], sq[:, :])
            arin = dr.tile([128, w, 2], F32, tag=f"arin{ci}",
                           name=f"arin{ci}")
            arout = dr.tile([128, w, 2], F32, tag=f"arout{ci}",
                            name=f"arout{ci}")
            nc.scalar.dma_start(out=arin[:, :, :], in_=arloc[:, 0:w, :])
            nc.gpsimd.collective_compute(
                "AllReduce",
                OP.add,
                replica_groups=GROUPS,
                ins=[arin[:, :, :].opt()],
                outs=[arout[:, :, :].opt()],
            )
            arg = pm.tile([128, 4, 2], F32, tag="arg", bufs=2,
                          name=f"arg_{ci}")
            nc.sync.dma_start(out=arg[:, 0:w, :], in_=arout[:, 0:w, :])
            # mu = 0.25*S1; var = 0.25*S2 - mu^2; rs2 = 1/sqrt(var+eps)
            mu2 = pm.tile([128, 4], F32, tag="mu2", bufs=2, name=f"mu2_{ci}")
            rs2 = pm.tile([128, 4], F32, tag="rs2", bufs=2, name=f"rs2_{ci}")
            var2 = pm.tile([128, 4], F32, tag="var2", bufs=2)
            musq = pm.tile([128, 4], F32, tag="musq", bufs=2)
            nc.vector.tensor_scalar_mul(mu2[:, 0:w], arg[:, 0:w, 0], 0.25)
            nc.vector.tensor_mul(musq[:, 0:w], mu2[:, 0:w], mu2[:, 0:w])
            nc.vector.tensor_scalar_mul(var2[:, 0:w], arg[:, 0:w, 1], 0.25)
            nc.vector.tensor_sub(var2[:, 0:w], var2[:, 0:w], musq[:, 0:w])
            s2t = pm.tile([128, 4], F32, tag="s2t", bufs=2)
            nc.scalar.activation(s2t[:, 0:w], var2[:, 0:w], AF.Sqrt,
                                 bias=eps_sb[:, :])
            nc.vector.reciprocal(rs2[:, 0:w], s2t[:, 0:w])
            rsmu2 = pm.tile([128, 4], F32, tag="rsmu2", bufs=2,
                            name=f"rsmu2_{ci}")
            nc.vector.tensor_mul(rsmu2[:, 0:w], rs2[:, 0:w], mu2[:, 0:w])
            ln2_state[ci] = (mu2, rs2, rsmu2)
            if debug and ci == 0:
                nc.sync.dma_start(out=dbg_mu2[:, :], in_=mu2[:, :])
                nc.sync.dma_start(out=dbg_rs2[:, :], in_=rs2[:, :])

        # ---------------- row-parallel wo + ReduceScatter ----------------
        rs_done = {}

        def emit_ln2_wo(ci):
            s0, w = CHUNKS[ci]
            attn_c = attn_tiles.pop(ci)
            osb_tiles = []
            for j in range(w):
                t = s0 + j
                aT = pm.tile([128, 4, 128], BF16, tag="aT", bufs=2,
                             name=f"aT_{t}")
                nc.sync.dma_start_transpose(aT[:, :, :], attn_c[:, j, :])
                osb = pm.tile([128, DIM], BF16, tag="osb", bufs=2,
                              name=f"osb_{t}")
                for half in range(2):
                    po = ps.tile([128, 1024], F32, tag="mega", bufs=2,
                                 name=f"po_{t}_{half}")
                    for oc2 in range(2):
                        colr = 1024 * half + 512 * oc2
                        for c in range(4):
                            nc.tensor.matmul(
                                po[:, 512 * oc2:512 * oc2 + 512],
                                lhsT=aT[:, c, :],
                                rhs=wo_sb[:, c, colr:colr + 512],
                                start=(c == 0), stop=(c == 3),
                                skip_group_check=True)
                    nc.vector.tensor_copy(
                        osb[:, 1024 * half:1024 * half + 1024], po[:, :])
                osb_tiles.append(osb)
                if debug and ci == 0 and j == 0:
                    nc.sync.dma_start(out=dbg_osb[:, :], in_=osb[:, :])
            # one ReduceScatter per chunk into a staging buffer, then out
            rin = dr.tile([4, 128 * w, DQ_LOC], BF16, tag=f"rin{s0}",
                          name=f"rin{s0}")
            for dj in range(w):
                nc.sync.dma_start(
                    out=rin[:, 128 * dj:128 * dj + 128, :]
                        .rearrange("g p n -> p g n"),
                    in_=osb_tiles[dj][:, :]
                        .rearrange("p (g n) -> p g n", g=4))
            rout = dr.tile([128 * w, DQ_LOC], BF16, tag=f"rout{s0}",
                           name=f"rout{s0}")
            nc.gpsimd.collective_compute(
                "ReduceScatter",
                OP.add,
                replica_groups=GROUPS,
                ins=[rin[:, :, :].opt()],
                outs=[rout[:, :].opt()],
            )
            rs_done[ci] = rout

        def emit_ln2_fix(ci):
            # post-RS LN2 fixup: out = rs2*praw - rsmu2*CWO + bias
            s0, w = CHUNKS[ci]
            rout = rs_done.pop(ci)
            mu2, rs2, rsmu2 = ln2_state.pop(ci)
            for j in range(w):
                t = s0 + j
                ofin = pm.tile([128, DQ_LOC], BF16, tag="ofin", bufs=2,
                               name=f"ofin_{t}")
                nc.scalar.dma_start(out=ofin[:, :],
                                    in_=rout[128 * j:128 * j + 128, :])
                atmp = pm.tile([128, DQ_LOC], BF16, tag="atmp", bufs=2,
                               name=f"atmp_{t}")
                nc.vector.scalar_tensor_tensor(
                    out=atmp[:, :], in0=cwos_sb[:, :],
                    scalar=rsmu2[:, j:j + 1], in1=wob_sb[:, :],
                    op0=OP.mult, op1=OP.subtract)
                nc.vector.scalar_tensor_tensor(
                    out=ofin[:, :], in0=ofin[:, :],
                    scalar=rs2[:, j:j + 1], in1=atmp[:, :],
                    op0=OP.mult, op1=OP.subtract)
                nc.scalar.dma_start(out=out[128 * t:128 * t + 128, :],
                                    in_=ofin[:, :])

        # ================= schedule =================
        emit_block_trans_stats(0)
        emit_qkv(0)
        emit_csum()
        for i in range(8, 12):
            load_xt(i)
        emit_attention(0, mids={1: (lambda: emit_block_trans_stats(1))},
                       post_attn=(lambda: emit_ln2_ar(0)))

        emit_qkv(1)
        emit_late_weights()
        for i in range(12, 16):
            load_xt(i)
        emit_attention(1, mids={1: (lambda: emit_block_trans_stats(2)),
                                3: (lambda: emit_ln2_wo(0))},
                       post_attn=(lambda: emit_ln2_ar(1)))

        emit_qkv(2)
        emit_attention(2, mids={1: (lambda: emit_block_trans_stats(3)),
                                3: (lambda: emit_ln2_wo(1))},
                       post_attn=(lambda: (emit_ln2_ar(2), emit_ln2_fix(0))))

        emit_qkv(3)
        emit_attention(3, mids={2: (lambda: emit_ln2_wo(2))},
                       post_attn=(lambda: (emit_ln2_ar(3), emit_ln2_fix(1))))
        emit_attention(4, mids={2: (lambda: emit_ln2_wo(3))},
                       post_attn=(lambda: (emit_ln2_ar(4), emit_ln2_fix(2))))
        emit_ln2_wo(4)
        emit_ln2_fix(3)
        emit_ln2_fix(4)
        if debug:
            nc.sync.dma_start(out=dbg_kT[:, :], in_=kT[:, :])
            nc.sync.dma_start(out=dbg_qT[:, :], in_=qT[:, 0, :])
            nc.sync.dma_start(out=dbg_vaug[:, :, :], in_=v_aug[:, :, :])

    nc.compile()
    return nc


def _prep_inputs(x, freqs_cis, seizure_labels, wq, wk, wv, wo,
                 ln1_w, ln1_b, ln2_w, ln2_b):
    bf16 = ml_dtypes.bfloat16
    cos = np.asarray(freqs_cis[..., 0], dtype=np.float32)  # [S, 64]
    sin = np.asarray(freqs_cis[..., 1], dtype=np.float32)
    cosT = np.ascontiguousarray(np.repeat(cos.T, 2, axis=0), dtype=bf16)
    sgn = np.where(np.arange(HEAD_DIM) % 2 == 0, -1.0, 1.0).astype(np.float32)
    sinT = np.ascontiguousarray(np.repeat(sin.T, 2, axis=0) * sgn[:, None],
                                dtype=bf16)
    ident = np.eye(128, dtype=bf16)
    identf = np.eye(128, dtype=np.float32)
    psw = np.zeros((128, 128), dtype=np.float32)
    idx = np.arange(128)
    psw[idx ^ 1, idx] = 1.0  # out[m, s] = in[m^1, s]
    psw = psw.astype(bf16)
    kk = np.arange(128)[:, None]
    qq = np.arange(128)[None, :]
    ctri_np = np.where(qq >= kk, 0.0, NEG_SCALED).astype(bf16)

    # LN affine folding (host-side): ln(x)@W.T = xhat@(W*w).T + b@W.T, with
    # xhat's (mu, 1/s) entering the device matmuls via K=2 aug rows.
    w1 = np.asarray(ln1_w, np.float64)
    b1 = np.asarray(ln1_b, np.float64)
    w2 = np.asarray(ln2_w, np.float64)
    b2 = np.asarray(ln2_b, np.float64)
    in_maps = []
    for cid in range(8):
        b, g = divmod(cid, 4)
        sl = slice(DQ_LOC * g, DQ_LOC * (g + 1))
        slh = slice(HEAD_DIM * g, HEAD_DIM * (g + 1))
        wq_s = np.asarray(wq[sl, :], np.float64) * w1[None, :]
        wk_s = np.asarray(wk[slh, :], np.float64) * w1[None, :]
        wv_s = np.asarray(wv[slh, :], np.float64) * w1[None, :]
        qb_v = b1 @ np.asarray(wq[sl, :], np.float64).T      # [512]
        kb_v = b1 @ np.asarray(wk[slh, :], np.float64).T     # [128]
        vb_v = b1 @ np.asarray(wv[slh, :], np.float64).T     # [128]
        # aug rows pair with (mu_hi, s_hi, mu_lo, s_lo) stat rows:
        # psum += (-colsum) x mu_row + bias x s_row (pre-rs scaling)
        def aug4(cneg, bias):
            return np.stack([cneg, bias, cneg, bias]).astype(bf16)
        augq_np = aug4(-wq_s.sum(axis=1), qb_v)
        augk_np = aug4(-wk_s.sum(axis=1), kb_v)
        augv_np = aug4(-wv_s.sum(axis=1), vb_v)
        # row-parallel wo: this core holds input-dims sl of wo
        woRP = np.asarray(wo[:, sl], np.float64) * w2[None, sl]  # [2048,512]
        wo_f = np.asarray(wo, np.float64) * np.asarray(w2, np.float64)[None, :]
        bias_full = np.asarray(wo, np.float64) @ b2              # [2048]
        cwo_full = wo_f.sum(axis=1)                              # [2048]
        in_maps.append({
            "xs": np.ascontiguousarray(x[b], dtype=bf16),
            "wqT": np.ascontiguousarray(wq_s.T, dtype=bf16),
            "wkT": np.ascontiguousarray(wk_s.T, dtype=bf16),
            "wvT": np.ascontiguousarray(wv_s.T, dtype=bf16),
            "woT": np.ascontiguousarray(woRP.T, dtype=bf16),
            "augq": np.ascontiguousarray(augq_np),
            "augk": np.ascontiguousarray(augk_np),
            "augv": np.ascontiguousarray(augv_np),
            "identf": identf,
            "wob": np.ascontiguousarray(
                np.tile(bias_full[sl].astype(np.float32), (128, 1)),
                dtype=bf16),
            "cwos": np.ascontiguousarray(
                np.tile(cwo_full[sl].astype(np.float32), (128, 1)),
                dtype=bf16),
            "labels": np.ascontiguousarray(seizure_labels[b], dtype=bf16),
            "cosT": cosT, "sinT": sinT,
            "ident": ident, "pswap": psw, "ctri": ctri_np,
        })
    return in_maps


def run(inputs, trace=False, trace_cores=None, debug=False):
    x = np.asarray(inputs["x"])
    mask = np.asarray(inputs["mask"])
    # this kernel specializes the additive mask to the causal prefill mask
    causal = np.where(np.tril(np.ones((S, S), dtype=bool)), 0.0, NEG_INF
                      ).astype(np.float32)
    if not np.array_equal(mask, causal):
        raise NotImplementedError("kernel specialized for causal prefill mask")

    in_maps = _prep_inputs(
        x, np.asarray(inputs["freqs_cis"]), np.asarray(inputs["seizure_labels"]),
        np.asarray(inputs["wq"]), np.asarray(inputs["wk"]),
        np.asarray(inputs["wv"]), np.asarray(inputs["wo"]),
        np.asarray(inputs["ln1_w"]), np.asarray(inputs["ln1_b"]),
        np.asarray(inputs["ln2_w"]), np.asarray(inputs["ln2_b"]))

    key = f"nc{int(debug)}"
    if key not in _CACHED:
        _CACHED[key] = build_nc(debug=debug)
    nc = _CACHED[key]

    kw = {}
    if trace:
        kw = dict(trace=True,
                  trace_cores=trace_cores if trace_cores is not None else [0])
    res = run_bass_kernel_spmd(nc, in_maps, core_ids=list(range(8)), **kw)

    shards = [res.results[cid]["out"] for cid in range(8)]
    full = np.empty((B, S, DIM), dtype=np.float32)
    for cid in range(8):
        b, g = divmod(cid, 4)
        full[b, :, DQ_LOC * g:DQ_LOC * (g + 1)] = np.asarray(
            shards[cid], dtype=np.float32)
    return full, res


def kernel(**inputs) -> np.ndarray:
    out, _ = run(inputs, trace=False)
    return out


# revision 23
# speedup vs baseline: 1.0780x; 1.0780x over previous
"""Distributed Trainium2 Bass kernel for a dense-transformer attention block.

Sharding (8 NeuronCores): core cid = 4*b + g
  - b = batch index (B=2), g = kv-head group (N_KV_HEADS=4)
  - Each core: LN1(x[b]) -> its 4 query heads + its 1 kv head (column
    parallel wq/wk/wv), RoPE, causal GQA attention with pre-ictal bias,
    AllGather of per-group attention outputs (groups [0..3], [4..7]),
    LN2, column-parallel wo -> output columns [512g:512g+512].
  - Host concatenates the 8 output shards.

Schedule: 4 software-pipelined "columns" (512 seq rows each):
  col g4: LN1 stats/norm of rows -> XBAR DMA transpose into ln1T ->
  K/V/Q projections + RoPE for that seq block -> attention chunk g4
  (+ AllGather) -> LN2+wo for an earlier chunk whose gather landed.
Compute dtype: bf16 matmul operands, f32 PSUM accumulation, f32 softmax/LN.
"""

import math
from contextlib import ExitStack

import numpy as np
import ml_dtypes

import concourse.bass as bass
import concourse.bacc as bacc
import concourse.mybir as mybir
import concourse.tile as tile
from concourse.bass_utils import run_bass_kernel_spmd

# Problem constants (hardcoded per spec nn_Attention_36120674959366)
B = 2
S = 2048
DIM = 2048
N_HEADS = 16
N_KV_HEADS = 4
HEAD_DIM = 128
NH_LOC = N_HEADS // N_KV_HEADS  # 4 q-heads per core
DQ_LOC = NH_LOC * HEAD_DIM      # 512
PRE_ICTAL_WINDOW = 10
PRE_ICTAL_BIAS = 2.0
LN_EPS = 1e-5
NEG_INF = -1e9

SQD = math.sqrt(HEAD_DIM)           # 11.3137085
INV_SQD = 1.0 / SQD
BIAS_SCALED = PRE_ICTAL_BIAS * SQD  # 22.627417
NEG_SCALED = NEG_INF * SQD          # -1.13137085e10

NT = S // 128                        # 16 tiles of 128 rows
NC = DIM // 128                      # 16 dim chunks

F32 = mybir.dt.float32
BF16 = mybir.dt.bfloat16

# attention chunks of query tiles: (first_tile, n_tiles)
CHUNKS = [(0, 4), (4, 4), (8, 4), (12, 2), (14, 2)]

_CACHED = {}


def build_nc():
    nc = bacc.Bacc("TRN2", target_bir_lowering=False, debug=False, num_devices=8)

    # ---- kernel I/O (per-core shards; same graph on all 8 cores) ----
    xs = nc.dram_tensor("xs", [S, DIM], BF16, kind="ExternalInput")
    wqT = nc.dram_tensor("wqT", [DIM, DQ_LOC], BF16, kind="ExternalInput")
    wkT = nc.dram_tensor("wkT", [DIM, HEAD_DIM], BF16, kind="ExternalInput")
    wvT = nc.dram_tensor("wvT", [DIM, HEAD_DIM], BF16, kind="ExternalInput")
    woT = nc.dram_tensor("woT", [DIM, DQ_LOC], BF16, kind="ExternalInput")
    qb = nc.dram_tensor("qb", [128, NH_LOC], F32, kind="ExternalInput")
    kb = nc.dram_tensor("kb", [128, 1], F32, kind="ExternalInput")
    vbt = nc.dram_tensor("vbt", [128, HEAD_DIM], BF16, kind="ExternalInput")
    obt = nc.dram_tensor("obt", [128, DQ_LOC], F32, kind="ExternalInput")
    labels = nc.dram_tensor("labels", [S], BF16, kind="ExternalInput")
    cosT = nc.dram_tensor("cosT", [HEAD_DIM, S], BF16, kind="ExternalInput")
    sinT = nc.dram_tensor("sinT", [HEAD_DIM, S], BF16, kind="ExternalInput")
    ident = nc.dram_tensor("ident", [128, 128], BF16, kind="ExternalInput")
    pswap = nc.dram_tensor("pswap", [128, 128], BF16, kind="ExternalInput")
    ctri = nc.dram_tensor("ctri", [128, 128], BF16, kind="ExternalInput")
    out = nc.dram_tensor("out", [S, DQ_LOC], F32, kind="ExternalOutput")

    AF = mybir.ActivationFunctionType
    OP = mybir.AluOpType

    with tile.TileContext(nc) as tc, ExitStack() as st:
        pc = st.enter_context(tc.tile_pool(name="const", bufs=1))
        dr = st.enter_context(tc.tile_pool(name="dr", bufs=1, space="DRAM"))
        pw = st.enter_context(tc.tile_pool(name="wts", bufs=1))
        pm = st.enter_context(tc.tile_pool(name="work", bufs=1))
        ps = st.enter_context(tc.tile_pool(name="ps", bufs=1, space="PSUM"))

        # ======== priority DMA: first column's x tiles ========
        xt_tiles = {}

        def load_xt(i):
            xt = pm.tile([128, DIM], BF16, tag="xt", bufs=4, name=f"xt_{i}")
            for q in range(4):
                nc.sync.dma_start(
                    out=xt[:, 512 * q:512 * q + 512],
                    in_=xs[128 * i:128 * i + 128, 512 * q:512 * q + 512])
            xt_tiles[i] = xt

        for i in range(4):
            load_xt(i)

        # ======== constants / weights: allocations only; posts are
        # ordered by first-use around the LN1(0) chain ========
        ident_sb = pc.tile([128, 128], BF16, tag="ident")
        pswap_sb = pc.tile([128, 128], BF16, tag="pswap")
        ctri_sb = pc.tile([128, 128], BF16, tag="ctri")
        cos_sb = pc.tile([128, S], BF16, tag="cos")
        sin_sb = pc.tile([128, S], BF16, tag="sin")
        qb_sb = pc.tile([128, NH_LOC], F32, tag="qb")
        kb_sb = pc.tile([128, 1], F32, tag="kb")
        vb_sb = pc.tile([128, HEAD_DIM], BF16, tag="vb")
        wq_sb = pw.tile([128, NC, DQ_LOC], BF16, tag="wq")
        wk_sb = pw.tile([128, NC, HEAD_DIM], BF16, tag="wk")
        wv_sb = pw.tile([128, NC, HEAD_DIM], BF16, tag="wv")
        wo_sb = pw.tile([128, NC, DQ_LOC], BF16, tag="wo")
        ob_sb = pc.tile([128, DQ_LOC], F32, tag="ob")

        def emit_early_posts():
            # inputs needed by col0 K/V projections; issued before the
            # LN1(0) sqrt so the scalar queue reaches it promptly
            for q in range(2):
                nc.scalar.dma_start(
                    out=wk_sb[:, 8 * q:8 * q + 8, :],
                    in_=wkT.ap().rearrange("(c p) n -> p c n", p=128)
                        [:, 8 * q:8 * q + 8, :])
            for q in range(2):
                nc.scalar.dma_start(
                    out=wv_sb[:, 8 * q:8 * q + 8, :],
                    in_=wvT.ap().rearrange("(c p) n -> p c n", p=128)
                        [:, 8 * q:8 * q + 8, :])

        def emit_late_posts():
            nc.scalar.dma_start(out=pswap_sb[:, :], in_=pswap[:, :])
            nc.scalar.dma_start(out=cos_sb[:, :], in_=cosT[:, :])
            nc.scalar.dma_start(out=sin_sb[:, :], in_=sinT[:, :])
            nc.scalar.dma_start(out=kb_sb[:, :], in_=kb[:, :])
            nc.scalar.dma_start(out=qb_sb[:, :], in_=qb[:, :])
            nc.scalar.dma_start(out=vb_sb[:, :], in_=vbt[:, :])
            for q in range(4):
                nc.scalar.dma_start(
                    out=wq_sb[:, 4 * q:4 * q + 4, :],
                    in_=wqT.ap().rearrange("(c p) n -> p c n", p=128)
                        [:, 4 * q:4 * q + 4, :])
            nc.scalar.dma_start(out=ctri_sb[:, :], in_=ctri[:, :])
            nc.scalar.dma_start(out=ident_sb[:, :], in_=ident[:, :])

        def emit_wo_posts():
            for q in range(4):
                nc.scalar.dma_start(
                    out=wo_sb[:, 4 * q:4 * q + 4, :],
                    in_=woT.ap().rearrange("(c p) n -> p c n", p=128)
                        [:, 4 * q:4 * q + 4, :])
            nc.scalar.dma_start(out=ob_sb[:, :], in_=obt[:, :])

        eps_sb = pc.tile([128, 1], F32, tag="eps")
        nc.vector.memset(eps_sb[:, :], LN_EPS)
        ones_col = pc.tile([1, 128], BF16, tag="ones_col")
        nc.vector.memset(ones_col[:, :], 1.0)

        # LN statistics tiles (absolute-tile indexed)
        mv_all = pc.tile([128, NT, 2], F32, tag="mv_all")
        s_all = pc.tile([128, NT], F32, tag="s_all")
        rs_all = pc.tile([128, NT], F32, tag="rs_all")
        mv2_all = pc.tile([128, NT, 2], F32, tag="mv2_all")
        s2_all = pc.tile([128, NT], F32, tag="s2_all")
        rs2_all = pc.tile([128, NT], F32, tag="rs2_all")

        # ======== seizure-label cumulative sums -> colv/rb ========
        colv_bf = pc.tile([128, NT], BF16, tag="colv_bf")
        colv = pc.tile([128, NT], F32, tag="colv")
        rb_sb = pc.tile([128, S], BF16, tag="rb_sb")

        pcs = tc.alloc_tile_pool(name="csum", bufs=1)
        lab_sb = pcs.tile([1, S], BF16, tag="lab")

        def emit_csum():
            # cumulative seizure counts are small ints -> bf16 exact
            csrow_bf = pcs.tile([1, S + 12], BF16, tag="csrow_bf")
            nc.gpsimd.memset(csrow_bf[:, 0:1], 0.0)
            nc.vector.tensor_tensor_scan(
                out=csrow_bf[:, 1:S + 1],
                data0=lab_sb[:, :],
                data1=lab_sb[:, :],
                initial=0.0,
                op0=OP.add,
                op1=OP.bypass,
            )
            for j in range(11):
                nc.gpsimd.tensor_copy(csrow_bf[:, S + 1 + j:S + 2 + j],
                                      csrow_bf[:, S:S + 1])

            # colv[p, kt] = cs[min(128*kt + p + 10, S-1)] via DRAM bounce +
            # XBAR transpose (bf16 exact for small counts)
            csbuf = dr.tile([S + 12], BF16, tag="csbuf")
            nc.sync.dma_start(out=csbuf.rearrange("(o s) -> o s", o=1),
                              in_=csrow_bf[:, :])
            cs16 = pcs.tile([NT, 128], BF16, tag="cs16")
            nc.sync.dma_start(
                out=cs16[:, :],
                in_=csbuf[11:11 + S].rearrange("(t p) -> t p", p=128),
            )
            nc.sync.dma_start_transpose(colv_bf[:, :], cs16[:, :])
            nc.gpsimd.tensor_copy(colv[:, :], colv_bf[:, :])

            # rb_sb[p, q] = csrow[q] broadcast down partitions (cnt_a source)
            for sgm in range(4):
                rbp = ps.tile([128, 1024], F32, tag="mega", bufs=2,
                              name=f"rbp_{sgm}")
                nc.tensor.matmul(rbp[:, 0:512], lhsT=ones_col[:, :],
                                 rhs=csrow_bf[:, 512 * sgm:512 * sgm + 512],
                                 start=True, stop=True)
                nc.vector.tensor_copy(rb_sb[:, 512 * sgm:512 * sgm + 512],
                                      rbp[:, 0:512])
            pcs.release()

        # ======== persistent QKV tiles ========
        qT = pm.tile([128, NH_LOC, S], BF16, tag="qT")
        kT = pm.tile([128, S], BF16, tag="kT")
        v_aug = pm.tile([128, NT, 132], BF16, tag="v_aug")
        nc.gpsimd.memset(v_aug[:, :, 128:129], 1.0)

        bounce_outs = {}
        afs_tiles = {}

        # ---------------- helpers ----------------
        def emit_ln1_pipe(g4, ln1T):
            # per-tile stats -> sqrt -> norm -> XBAR transpose chains so the
            # first transpose fires ~15us earlier than a batched-sqrt ladder
            for j4 in range(4):
                i = 4 * g4 + j4
                xt = xt_tiles.pop(i)
                st6 = pm.tile([128, 4, 6], F32, tag="st6a", bufs=2)
                for a4 in range(4):
                    nc.vector.bn_stats(st6[:, a4, :],
                                       xt[:, 512 * a4:512 * a4 + 512])
                nc.vector.bn_aggr(mv_all[:, i, :], st6[:, :, :])
                nc.scalar.activation(s_all[:, i:i + 1], mv_all[:, i, 1:2],
                                     AF.Sqrt, bias=eps_sb[:, :])
                nc.vector.reciprocal(rs_all[:, i:i + 1], s_all[:, i:i + 1])
                xh = pm.tile([128, DIM], BF16, tag="xh", bufs=2)
                nc.vector.tensor_scalar(
                    out=xh[:, :], in0=xt[:, :],
                    scalar1=mv_all[:, i, 0:1], scalar2=rs_all[:, i:i + 1],
                    op0=OP.subtract, op1=OP.mult)
                nc.sync.dma_start_transpose(
                    ln1T[:, :, 128 * j4:128 * j4 + 128], xh[:, :])

        def rope_sg(dst_sg, w_sb, h, sg, ln1T):
            bias_ap = kb_sb[:, 0:1] if h is None else qb_sb[:, h:h + 1]
            pq = ps.tile([128, 1024], F32, tag="mega", bufs=2,
                         name=f"pq_{h}_{sg}")
            for c in range(NC):
                lhsT = w_sb[:, c, :] if h is None \
                    else w_sb[:, c, 128 * h:128 * h + 128]
                nc.tensor.matmul(pq[:, 0:512], lhsT=lhsT,
                                 rhs=ln1T[:, c, 0:512],
                                 start=(c == 0), stop=(c == NC - 1))
            raw = pm.tile([128, 512], BF16, tag="rope_raw", bufs=2)
            nc.scalar.activation(raw[:, :], pq[:, 0:512], AF.Identity,
                                 bias=bias_ap)
            pw2 = ps.tile([128, 1024], F32, tag="mega", bufs=2,
                          name=f"pw2_{h}_{sg}")
            nc.tensor.matmul(pw2[:, 0:512], lhsT=pswap_sb[:, :],
                             rhs=raw[:, :], start=True, stop=True)
            t1 = pm.tile([128, 512], BF16, tag="rope_t1", bufs=2)
            nc.vector.tensor_mul(t1[:, :], raw[:, :],
                                 cos_sb[:, 512 * sg:512 * sg + 512])
            t2 = pm.tile([128, 512], BF16, tag="rope_t2", bufs=2)
            nc.vector.tensor_mul(t2[:, :], pw2[:, 0:512],
                                 sin_sb[:, 512 * sg:512 * sg + 512])
            nc.vector.tensor_add(dst_sg, t1[:, :], t2[:, :])

        def emit_qkv(g4, ln1T):
            sg = g4
            # K projection + rope for this seq block
            rope_sg(kT[:, 512 * sg:512 * sg + 512], wk_sb, None, sg, ln1T)
            # V projection for the 4 seq tiles of this block
            vps = ps.tile([128, 1024], F32, tag="mega", bufs=2,
                          name=f"vps_{g4}")
            for j4 in range(4):
                i = 4 * g4 + j4
                reg = vps[:, 128 * j4:128 * j4 + 128]
                for c in range(NC):
                    nc.tensor.matmul(
                        reg, lhsT=ln1T[:, c, 128 * j4:128 * j4 + 128],
                        rhs=wv_sb[:, c, :],
                        start=(c == 0), stop=(c == NC - 1),
                        skip_group_check=True)
            for j4 in range(4):
                i = 4 * g4 + j4
                nc.vector.tensor_add(v_aug[:, i, 0:128],
                                     vps[:, 128 * j4:128 * j4 + 128],
                                     vb_sb[:, :])
            # Q projections + rope
            for h in range(NH_LOC):
                rope_sg(qT[:, h, 512 * sg:512 * sg + 512], wq_sb, h, sg, ln1T)

        def build_mg(ci):
            s0, w = CHUNKS[ci]
            kts = list(range(max(0, s0 - 1), s0 + w))
            mg = pm.tile([128, 5, 512], BF16, tag="mg", bufs=1,
                         name=f"mg_{ci}")
            nc.vector.memset(mg[:, :, :], 0.0)
            mg_idx = {}
            for slot, kt in enumerate(kts):
                mg_idx[kt] = slot
                if s0 <= kt < s0 + w:  # diag: qtile t == kt
                    j = kt - s0
                    sl = mg[:, slot, 128 * j:128 * j + 128]
                    nc.vector.tensor_scalar(
                        out=sl, in0=rb_sb[:, 128 * kt:128 * kt + 128],
                        scalar1=colv[:, kt:kt + 1], scalar2=BIAS_SCALED,
                        op0=OP.is_lt, op1=OP.mult,
                    )
                    nc.vector.tensor_add(sl, sl, ctri_sb[:, :])
                tprev = kt + 1
                if s0 <= tprev < s0 + w:  # prev: qtile t == kt + 1
                    j = tprev - s0
                    sl = mg[:, slot, 128 * j:128 * j + 128]
                    nc.vector.tensor_scalar(
                        out=sl, in0=rb_sb[:, 128 * tprev:128 * tprev + 128],
                        scalar1=colv[:, kt:kt + 1], scalar2=BIAS_SCALED,
                        op0=OP.is_lt, op1=OP.mult,
                    )
            return mg, mg_idx

        def emit_attention(ci, mids=None, post_attn=None):
            s0, w = CHUNKS[ci]
            bounce_outs[ci] = dr.tile([4, 128 * w, DQ_LOC], BF16,
                                      name=f"bout{ci}", tag=f"bout{ci}")
            mg, mg_idx = build_mg(ci)
            attn_c = pm.tile([128, 4, DQ_LOC], BF16, tag="attn_c",
                             bufs=2, name=f"attn_c{ci}")
            for h in range(NH_LOC):
                if mids is not None and h in mids:
                    mids[h]()
                nkt = s0 + w
                pvp = [ps.tile([128, 132], F32, tag="pv_acc", bufs=4,
                               name=f"pv_{h}_{ci}_{jj}")
                       for jj in range(w)]
                kt = 0
                while kt < nkt:
                    mega = ps.tile([128, 1024], F32, tag="mega", bufs=2,
                                   name=f"sc_{h}_{ci}_{kt}")
                    pair = [k2 for k2 in (kt, kt + 1) if k2 < nkt]
                    offs = []
                    for slot, k2 in enumerate(pair):
                        off = 128 * max(0, k2 - s0)
                        offs.append(off)
                        reg = mega[:, 512 * slot + off:
                                   512 * slot + 128 * w]
                        if k2 in mg_idx:
                            nc.tensor.matmul(
                                reg, lhsT=ident_sb[:, :],
                                rhs=mg[:, mg_idx[k2], off:128 * w],
                                start=True, stop=False)
                            nc.tensor.matmul(
                                reg,
                                lhsT=kT[:, 128 * k2:128 * k2 + 128],
                                rhs=qT[:, h, 128 * s0 + off:
                                       128 * (s0 + w)],
                                start=False, stop=True)
                        else:
                            nc.tensor.matmul(
                                reg,
                                lhsT=kT[:, 128 * k2:128 * k2 + 128],
                                rhs=qT[:, h, 128 * s0 + off:
                                       128 * (s0 + w)],
                                start=True, stop=True)
                    pt = pm.tile([128, 1024], BF16, tag="pt_sm", bufs=2)
                    if len(pair) == 2 and offs[0] == 0 and offs[1] == 0:
                        if w == 4:
                            nc.scalar.activation(pt[:, :], mega[:, :],
                                                 AF.Exp, scale=INV_SQD)
                        else:
                            for slot in range(2):
                                nc.scalar.activation(
                                    pt[:, 512 * slot:512 * slot + 128 * w],
                                    mega[:, 512 * slot:512 * slot + 128 * w],
                                    AF.Exp, scale=INV_SQD)
                    else:
                        for slot, k2 in enumerate(pair):
                            off = offs[slot]
                            nc.scalar.activation(
                                pt[:, 512 * slot + off:512 * slot + 128 * w],
                                mega[:, 512 * slot + off:512 * slot + 128 * w],
                                AF.Exp, scale=INV_SQD)
                    for slot, k2 in enumerate(pair):
                        for j in range(max(0, k2 - s0), w):
                            nc.tensor.matmul(
                                pvp[j][:, 0:129],
                                lhsT=pt[:, 512 * slot + 128 * j:
                                        512 * slot + 128 * j + 128],
                                rhs=v_aug[:, k2, 0:129],
                                start=(k2 == 0), stop=(k2 == s0 + j),
                                skip_group_check=True)
                    kt += len(pair)
                for j in range(w):
                    rcp = pm.tile([128, 1], F32, tag="rcp", bufs=4,
                                  name=f"rcp_{h}_{ci}_{j}")
                    nc.vector.reciprocal(rcp[:, :], pvp[j][:, 128:129])
                    nc.vector.tensor_single_scalar(
                        out=attn_c[:, j, 128 * h:128 * h + 128],
                        in_=pvp[j][:, 0:128],
                        scalar=rcp[:, :],
                        op=OP.mult)

            bounce_in = dr.tile([128 * w, DQ_LOC], BF16, name=f"bin{ci}",
                                tag=f"bin{ci}")
            bounce_out = bounce_outs[ci]
            nc.sync.dma_start(
                out=bounce_in.rearrange("(t p) n -> p t n", p=128),
                in_=attn_c[:, 0:w, :])
            nc.gpsimd.collective_compute(
                "AllGather",
                mybir.AluOpType.bypass,
                replica_groups=[[0, 1, 2, 3], [4, 5, 6, 7]],
                ins=[bounce_in[:, :].opt()],
                outs=[bounce_out[:, :, :].opt()],
            )
            if post_attn is not None:
                post_attn()

        def emit_ln2_stats(ci):
            s0, w = CHUNKS[ci]
            for j in range(w):
                t = s0 + j
                afs = pm.tile([128, 4, 512], BF16, tag="afs", bufs=4,
                              name=f"afs_{t}")
                nc.sync.dma_start(
                    out=afs[:, :, :],
                    in_=bounce_outs[ci][:, 128 * j:128 * j + 128, :]
                        .rearrange("g p n -> p g n"))
                afs_tiles[t] = afs
                st6b = pm.tile([128, 4, 6], F32, tag="st6b", bufs=2)
                for a4 in range(4):
                    nc.vector.bn_stats(st6b[:, a4, :], afs[:, a4, :])
                nc.vector.bn_aggr(mv2_all[:, t, :], st6b[:, :, :])

        def emit_ln2_sqrt(ci):
            s0, w = CHUNKS[ci]
            sl = slice(s0, s0 + w)
            nc.scalar.activation(s2_all[:, sl], mv2_all[:, sl, 1:2], AF.Sqrt,
                                 bias=eps_sb[:, :])
            nc.vector.reciprocal(rs2_all[:, sl], s2_all[:, sl])

        def emit_ln2_mm(ci):
            s0, w = CHUNKS[ci]
            for j in range(w):
                t = s0 + j
                afs = afs_tiles.pop(t)
                xh2 = pm.tile([128, DIM], BF16, tag="xh2", bufs=2,
                              name=f"xh2_{t}")
                nc.vector.tensor_scalar(
                    out=xh2[:, :],
                    in0=afs.rearrange("p g n -> p (g n)"),
                    scalar1=mv2_all[:, t, 0:1], scalar2=rs2_all[:, t:t + 1],
                    op0=OP.subtract, op1=OP.mult)
                ln2T = pm.tile([128, NC, 128], BF16, tag="ln2T", bufs=2,
                               name=f"ln2T_{t}")
                nc.sync.dma_start_transpose(ln2T[:, :, :], xh2[:, :])
                po = ps.tile([128, 1024], F32, tag="mega", bufs=2,
                             name=f"po_{t}")
                for c in range(NC):
                    nc.tensor.matmul(
                        po[:, 0:512],
                        lhsT=ln2T[:, c, :],
                        rhs=wo_sb[:, c, :],
                        start=(c == 0), stop=(c == NC - 1))
                osb = pm.tile([128, DQ_LOC], F32, tag="osb", bufs=2,
                              name=f"osb_{t}")
                nc.vector.tensor_add(osb[:, :], po[:, 0:512], ob_sb[:, :])
                nc.sync.dma_start(out=out[128 * t:128 * t + 128, :],
                                   in_=osb[:, :])

        # ================= pipelined columns =================
        # col g4: [prefetch x(g4+1)] LN1(g4) QKV(g4) [ln2 stats]
        #         ATTN(chunk) [+AG] [ln2 mm]
        def emit_ln1_block(g4):
            # full LN1 chain for column g4; DMA lands before any later
            # collective parks the queues.
            if g4 < 4:
                ln1T = pm.tile([128, NC, 512], BF16, tag="ln1T", bufs=2,
                               name=f"ln1T_{g4}")
                emit_ln1_pipe(g4, ln1T)
                ln1T_tiles[g4] = ln1T

        def emit_ln2_block(ci):
            emit_ln2_stats(ci)
            emit_ln2_sqrt(ci)
            emit_ln2_mm(ci)

        def emit_ln2_tail(ci):
            # per-tile pipelined chain; emitted BEFORE the chunk's bounce/AG
            # so every DMA is queued ahead of the collective's descriptors
            s0, w = CHUNKS[ci]
            for j in range(w):
                t = s0 + j
                afs = pm.tile([128, 4, 512], BF16, tag="afs", bufs=4,
                              name=f"afs_{t}")
                nc.sync.dma_start(
                    out=afs[:, :, :],
                    in_=bounce_outs[ci][:, 128 * j:128 * j + 128, :]
                        .rearrange("g p n -> p g n"))
                st6b = pm.tile([128, 4, 6], F32, tag="st6b", bufs=2)
                for a4 in range(4):
                    nc.vector.bn_stats(st6b[:, a4, :], afs[:, a4, :])
                nc.vector.bn_aggr(mv2_all[:, t, :], st6b[:, :, :])
                nc.scalar.activation(s2_all[:, t:t + 1], mv2_all[:, t, 1:2],
                                     AF.Sqrt, bias=eps_sb[:, :])
                nc.vector.reciprocal(rs2_all[:, t:t + 1], s2_all[:, t:t + 1])
                xh2 = pm.tile([128, DIM], BF16, tag="xh2", bufs=2,
                              name=f"xh2_{t}")
                nc.vector.tensor_scalar(
                    out=xh2[:, :],
                    in0=afs.rearrange("p g n -> p (g n)"),
                    scalar1=mv2_all[:, t, 0:1], scalar2=rs2_all[:, t:t + 1],
                    op0=OP.subtract, op1=OP.mult)
                ln2T = pm.tile([128, NC, 128], BF16, tag="ln2T", bufs=2,
                               name=f"ln2T_{t}")
                nc.sync.dma_start_transpose(ln2T[:, :, :], xh2[:, :])
                po = ps.tile([128, 1024], F32, tag="mega", bufs=2,
                             name=f"po_{t}")
                for c in range(NC):
                    nc.tensor.matmul(
                        po[:, 0:512],
                        lhsT=ln2T[:, c, :],
                        rhs=wo_sb[:, c, :],
                        start=(c == 0), stop=(c == NC - 1))
                osb = pm.tile([128, DQ_LOC], F32, tag="osb", bufs=2,
                              name=f"osb_{t}")
                nc.vector.tensor_add(osb[:, :], po[:, 0:512], ob_sb[:, :])
                nc.sync.dma_start(out=out[128 * t:128 * t + 128, :],
                                  in_=osb[:, :])

        ln1T_tiles = {}
        nc.scalar.dma_start(out=lab_sb[:, :],
                            in_=labels.ap().rearrange("(o s) -> o s", o=1))
        emit_early_posts()
        emit_ln1_block(0)
        emit_late_posts()

        for i in range(4, 8):
            load_xt(i)
        emit_qkv(0, ln1T_tiles.pop(0))
        emit_csum()
        emit_attention(0, mids={1: (lambda: emit_ln1_block(1))})

        for i in range(8, 12):
            load_xt(i)
        emit_qkv(1, ln1T_tiles.pop(1))
        emit_wo_posts()
        emit_attention(1, mids={1: (lambda: emit_ln1_block(2))})

        for i in range(12, 16):
            load_xt(i)
        emit_qkv(2, ln1T_tiles.pop(2))
        emit_attention(2, mids={1: (lambda: emit_ln1_block(3)),
                                2: (lambda: emit_ln2_block(0))})

        emit_qkv(3, ln1T_tiles.pop(3))
        emit_attention(3, mids={1: (lambda: emit_ln2_block(1)),
                                2: (lambda: emit_ln2_block(2))})
        emit_attention(4, mids={2: (lambda: emit_ln2_block(3))},
                       post_attn=(lambda: emit_ln2_tail(4)))

    nc.compile()
    return nc


def _prep_inputs(x, freqs_cis, seizure_labels, wq, wk, wv, wo,
                 ln1_w, ln1_b, ln2_w, ln2_b):
    bf16 = ml_dtypes.bfloat16
    cos = np.asarray(freqs_cis[..., 0], dtype=np.float32)  # [S, 64]
    sin = np.asarray(freqs_cis[..., 1], dtype=np.float32)
    cosT = np.ascontiguousarray(np.repeat(cos.T, 2, axis=0), dtype=bf16)
    sgn = np.where(np.arange(HEAD_DIM) % 2 == 0, -1.0, 1.0).astype(np.float32)
    sinT = np.ascontiguousarray(np.repeat(sin.T, 2, axis=0) * sgn[:, None],
                                dtype=bf16)
    ident = np.eye(128, dtype=bf16)
    psw = np.zeros((128, 128), dtype=np.float32)
    idx = np.arange(128)
    psw[idx ^ 1, idx] = 1.0  # out[m, s] = sum_k psw[k, m] * in[k, s] = in[m^1, s]
    psw = psw.astype(bf16)
    kk = np.arange(128)[:, None]
    qq = np.arange(128)[None, :]
    ctri_np = np.where(qq >= kk, 0.0, NEG_SCALED).astype(bf16)

    # fold LN affine weights into the projection weights (host-side
    # preprocessing, standard inference-time weight folding):
    #   ln(x)@W.T = xhat@(W*w).T + b@W.T
    w1 = np.asarray(ln1_w, np.float64)
    b1 = np.asarray(ln1_b, np.float64)
    w2 = np.asarray(ln2_w, np.float64)
    b2 = np.asarray(ln2_b, np.float64)
    in_maps = []
    for cid in range(8):
        b, g = divmod(cid, 4)
        wq_s = np.asarray(wq[DQ_LOC * g:DQ_LOC * (g + 1), :], np.float64)
        wk_s = np.asarray(wk[HEAD_DIM * g:HEAD_DIM * (g + 1), :], np.float64)
        wv_s = np.asarray(wv[HEAD_DIM * g:HEAD_DIM * (g + 1), :], np.float64)
        wo_s = np.asarray(wo[DQ_LOC * g:DQ_LOC * (g + 1), :], np.float64)
        qb_v = (b1 @ wq_s.T).astype(np.float32)         # [512]
        kb_v = (b1 @ wk_s.T).astype(np.float32)         # [128]
        vb_v = (b1 @ wv_s.T).astype(np.float32)         # [128]
        ob_v = (b2 @ wo_s.T).astype(np.float32)         # [512]
        in_maps.append({
            "xs": np.ascontiguousarray(x[b], dtype=bf16),
            "wqT": np.ascontiguousarray((wq_s * w1).T, dtype=bf16),
            "wkT": np.ascontiguousarray((wk_s * w1).T, dtype=bf16),
            "wvT": np.ascontiguousarray((wv_s * w1).T, dtype=bf16),
            "woT": np.ascontiguousarray((wo_s * w2).T, dtype=bf16),
            "qb": np.ascontiguousarray(
                qb_v.reshape(NH_LOC, 128).T, dtype=np.float32),
            "kb": np.ascontiguousarray(kb_v.reshape(128, 1), dtype=np.float32),
            "vbt": np.ascontiguousarray(np.tile(vb_v, (128, 1)), dtype=bf16),
            "obt": np.ascontiguousarray(np.tile(ob_v, (128, 1)),
                                        dtype=np.float32),
            "labels": np.ascontiguousarray(seizure_labels[b], dtype=bf16),
            "cosT": cosT, "sinT": sinT,
            "ident": ident, "pswap": psw, "ctri": ctri_np,
        })
    return in_maps


def run(inputs, trace=False, trace_cores=None):
    x = np.asarray(inputs["x"])
    mask = np.asarray(inputs["mask"])
    # this kernel specializes the additive mask to the causal prefill mask
    causal = np.where(np.tril(np.ones((S, S), dtype=bool)), 0.0, NEG_INF
                      ).astype(np.float32)
    if not np.array_equal(mask, causal):
        raise NotImplementedError("kernel specialized for causal prefill mask")

    in_maps = _prep_inputs(
        x, np.asarray(inputs["freqs_cis"]), np.asarray(inputs["seizure_labels"]),
        np.asarray(inputs["wq"]), np.asarray(inputs["wk"]),
        np.asarray(inputs["wv"]), np.asarray(inputs["wo"]),
        np.asarray(inputs["ln1_w"]), np.asarray(inputs["ln1_b"]),
        np.asarray(inputs["ln2_w"]), np.asarray(inputs["ln2_b"]))

    if "nc" not in _CACHED:
        _CACHED["nc"] = build_nc()
    nc = _CACHED["nc"]

    kw = {}
    if trace:
        kw = dict(trace=True,
                  trace_cores=trace_cores if trace_cores is not None else [0])
    res = run_bass_kernel_spmd(nc, in_maps, core_ids=list(range(8)), **kw)

    shards = [res.results[cid]["out"] for cid in range(8)]
    full = np.empty((B, S, DIM), dtype=np.float32)
    for cid in range(8):
        b, g = divmod(cid, 4)
        full[b, :, DQ_LOC * g:DQ_LOC * (g + 1)] = shards[cid]
    return full, res


def kernel(**inputs) -> np.ndarray:
    out, _ = run(inputs, trace=False)
    return out



# revision 24
# speedup vs baseline: 1.2187x; 1.1305x over previous
"""Distributed Trainium2 Bass kernel for a dense-transformer attention block.

Sharding (8 NeuronCores): core cid = 4*b + g
  - b = batch index (B=2), g = kv-head group (N_KV_HEADS=4)
  - Each core: LN1(x[b]) -> its 4 query heads + its 1 kv head (column
    parallel wq/wk/wv), RoPE, causal GQA attention with pre-ictal bias,
    AllGather of per-group attention outputs (groups [0..3], [4..7]),
    LN2, column-parallel wo -> output columns [512g:512g+512].
  - Host concatenates the 8 output shards.

Schedule: 4 software-pipelined "columns" (512 seq rows each):
  col g4: LN1 stats/norm of rows -> XBAR DMA transpose into ln1T ->
  K/V/Q projections + RoPE for that seq block -> attention chunk g4
  (+ AllGather) -> LN2+wo for an earlier chunk whose gather landed.
Compute dtype: bf16 matmul operands, f32 PSUM accumulation, f32 softmax/LN.
"""

import math
from contextlib import ExitStack

import numpy as np
import ml_dtypes

import concourse.bass as bass
import concourse.bacc as bacc
import concourse.mybir as mybir
import concourse.tile as tile
from concourse.bass_utils import run_bass_kernel_spmd

# Problem constants (hardcoded per spec nn_Attention_36120674959366)
B = 2
S = 2048
DIM = 2048
N_HEADS = 16
N_KV_HEADS = 4
HEAD_DIM = 128
NH_LOC = N_HEADS // N_KV_HEADS  # 4 q-heads per core
DQ_LOC = NH_LOC * HEAD_DIM      # 512
PRE_ICTAL_WINDOW = 10
PRE_ICTAL_BIAS = 2.0
LN_EPS = 1e-5
NEG_INF = -1e9

SQD = math.sqrt(HEAD_DIM)           # 11.3137085
INV_SQD = 1.0 / SQD
BIAS_SCALED = PRE_ICTAL_BIAS * SQD  # 22.627417
NEG_SCALED = NEG_INF * SQD          # -1.13137085e10

NT = S // 128                        # 16 tiles of 128 rows
NC = DIM // 128                      # 16 dim chunks

F32 = mybir.dt.float32
BF16 = mybir.dt.bfloat16

# attention chunks of query tiles: (first_tile, n_tiles)
CHUNKS = [(0, 4), (4, 4), (8, 4), (12, 4)]

_CACHED = {}


def build_nc():
    nc = bacc.Bacc("TRN2", target_bir_lowering=False, debug=False, num_devices=8)

    # ---- kernel I/O (per-core shards; same graph on all 8 cores) ----
    xs = nc.dram_tensor("xs", [S, DIM], BF16, kind="ExternalInput")
    wqT = nc.dram_tensor("wqT", [DIM, DQ_LOC], BF16, kind="ExternalInput")
    wkT = nc.dram_tensor("wkT", [DIM, HEAD_DIM], BF16, kind="ExternalInput")
    wvT = nc.dram_tensor("wvT", [DIM, HEAD_DIM], BF16, kind="ExternalInput")
    woT = nc.dram_tensor("woT", [DIM, DQ_LOC], BF16, kind="ExternalInput")
    qb = nc.dram_tensor("qb", [128, NH_LOC], F32, kind="ExternalInput")
    kb = nc.dram_tensor("kb", [128, 1], F32, kind="ExternalInput")
    vbt = nc.dram_tensor("vbt", [128, HEAD_DIM], BF16, kind="ExternalInput")
    obt = nc.dram_tensor("obt", [128, DQ_LOC], F32, kind="ExternalInput")
    labels = nc.dram_tensor("labels", [S], BF16, kind="ExternalInput")
    cosT = nc.dram_tensor("cosT", [HEAD_DIM, S], BF16, kind="ExternalInput")
    sinT = nc.dram_tensor("sinT", [HEAD_DIM, S], BF16, kind="ExternalInput")
    ident = nc.dram_tensor("ident", [128, 128], BF16, kind="ExternalInput")
    pswap = nc.dram_tensor("pswap", [128, 128], BF16, kind="ExternalInput")
    ctri = nc.dram_tensor("ctri", [128, 128], BF16, kind="ExternalInput")
    out = nc.dram_tensor("out", [S, DQ_LOC], F32, kind="ExternalOutput")

    AF = mybir.ActivationFunctionType
    OP = mybir.AluOpType

    with tile.TileContext(nc) as tc, ExitStack() as st:
        pc = st.enter_context(tc.tile_pool(name="const", bufs=1))
        dr = st.enter_context(tc.tile_pool(name="dr", bufs=1, space="DRAM"))
        pw = st.enter_context(tc.tile_pool(name="wts", bufs=1))
        pm = st.enter_context(tc.tile_pool(name="work", bufs=1))
        ps = st.enter_context(tc.tile_pool(name="ps", bufs=1, space="PSUM"))

        # ======== priority DMA: first column's x tiles ========
        xt_tiles = {}

        def load_xt(i):
            xt = pm.tile([128, DIM], BF16, tag="xt", bufs=4, name=f"xt_{i}")
            for q in range(4):
                nc.sync.dma_start(
                    out=xt[:, 512 * q:512 * q + 512],
                    in_=xs[128 * i:128 * i + 128, 512 * q:512 * q + 512])
            xt_tiles[i] = xt

        for i in range(4):
            load_xt(i)

        # ======== constants / weights: allocations only; posts are
        # ordered by first-use around the LN1(0) chain ========
        ident_sb = pc.tile([128, 128], BF16, tag="ident")
        pswap_sb = pc.tile([128, 128], BF16, tag="pswap")
        ctri_sb = pc.tile([128, 128], BF16, tag="ctri")
        cos_sb = pc.tile([128, S], BF16, tag="cos")
        sin_sb = pc.tile([128, S], BF16, tag="sin")
        qb_sb = pc.tile([128, NH_LOC], F32, tag="qb")
        kb_sb = pc.tile([128, 1], F32, tag="kb")
        vb_sb = pc.tile([128, HEAD_DIM], BF16, tag="vb")
        wq_sb = pw.tile([128, NC, DQ_LOC], BF16, tag="wq")
        wk_sb = pw.tile([128, NC, HEAD_DIM], BF16, tag="wk")
        wv_sb = pw.tile([128, NC, HEAD_DIM], BF16, tag="wv")
        wo_sb = pw.tile([128, NC, DQ_LOC], BF16, tag="wo")
        ob_sb = pc.tile([128, DQ_LOC], F32, tag="ob")

        def emit_early_posts():
            # inputs needed by col0 K/V projections; issued before the
            # LN1(0) sqrt so the scalar queue reaches it promptly
            for q in range(2):
                nc.scalar.dma_start(
                    out=wk_sb[:, 8 * q:8 * q + 8, :],
                    in_=wkT.ap().rearrange("(c p) n -> p c n", p=128)
                        [:, 8 * q:8 * q + 8, :])
            for q in range(2):
                nc.scalar.dma_start(
                    out=wv_sb[:, 8 * q:8 * q + 8, :],
                    in_=wvT.ap().rearrange("(c p) n -> p c n", p=128)
                        [:, 8 * q:8 * q + 8, :])

        def emit_late_posts():
            nc.scalar.dma_start(out=pswap_sb[:, :], in_=pswap[:, :])
            nc.scalar.dma_start(out=cos_sb[:, :], in_=cosT[:, :])
            nc.scalar.dma_start(out=sin_sb[:, :], in_=sinT[:, :])
            nc.scalar.dma_start(out=kb_sb[:, :], in_=kb[:, :])
            nc.scalar.dma_start(out=qb_sb[:, :], in_=qb[:, :])
            nc.scalar.dma_start(out=vb_sb[:, :], in_=vbt[:, :])
            for q in range(4):
                nc.scalar.dma_start(
                    out=wq_sb[:, 4 * q:4 * q + 4, :],
                    in_=wqT.ap().rearrange("(c p) n -> p c n", p=128)
                        [:, 4 * q:4 * q + 4, :])
            nc.scalar.dma_start(out=ctri_sb[:, :], in_=ctri[:, :])
            nc.scalar.dma_start(out=ident_sb[:, :], in_=ident[:, :])
            for q in range(4):
                nc.scalar.dma_start(
                    out=wo_sb[:, 4 * q:4 * q + 4, :],
                    in_=woT.ap().rearrange("(c p) n -> p c n", p=128)
                        [:, 4 * q:4 * q + 4, :])
            nc.scalar.dma_start(out=ob_sb[:, :], in_=obt[:, :])

        eps_sb = pc.tile([128, 1], F32, tag="eps")
        nc.vector.memset(eps_sb[:, :], LN_EPS)
        ones_col = pc.tile([1, 128], BF16, tag="ones_col")
        nc.vector.memset(ones_col[:, :], 1.0)

        # LN statistics tiles (absolute-tile indexed)
        mv_all = pc.tile([128, NT, 2], F32, tag="mv_all")
        s_all = pc.tile([128, NT], F32, tag="s_all")
        rs_all = pc.tile([128, NT], F32, tag="rs_all")
        mv2_all = pc.tile([128, NT, 2], F32, tag="mv2_all")
        s2_all = pc.tile([128, NT], F32, tag="s2_all")
        rs2_all = pc.tile([128, NT], F32, tag="rs2_all")

        # ======== seizure-label cumulative sums -> colv/rb ========
        colv_bf = pc.tile([128, NT], BF16, tag="colv_bf")
        colv = pc.tile([128, NT], F32, tag="colv")
        rb_sb = pc.tile([128, S], BF16, tag="rb_sb")

        pcs = tc.alloc_tile_pool(name="csum", bufs=1)
        lab_sb = pcs.tile([1, S], BF16, tag="lab")

        def emit_csum():
            # cumulative seizure counts are small ints -> bf16 exact
            csrow_bf = pcs.tile([1, S + 12], BF16, tag="csrow_bf")
            nc.gpsimd.memset(csrow_bf[:, 0:1], 0.0)
            nc.vector.tensor_tensor_scan(
                out=csrow_bf[:, 1:S + 1],
                data0=lab_sb[:, :],
                data1=lab_sb[:, :],
                initial=0.0,
                op0=OP.add,
                op1=OP.bypass,
            )
            for j in range(11):
                nc.gpsimd.tensor_copy(csrow_bf[:, S + 1 + j:S + 2 + j],
                                      csrow_bf[:, S:S + 1])

            # colv[p, kt] = cs[min(128*kt + p + 10, S-1)] via DRAM bounce +
            # XBAR transpose (bf16 exact for small counts)
            csbuf = dr.tile([S + 12], BF16, tag="csbuf")
            nc.sync.dma_start(out=csbuf.rearrange("(o s) -> o s", o=1),
                              in_=csrow_bf[:, :])
            cs16 = pcs.tile([NT, 128], BF16, tag="cs16")
            nc.sync.dma_start(
                out=cs16[:, :],
                in_=csbuf[11:11 + S].rearrange("(t p) -> t p", p=128),
            )
            nc.sync.dma_start_transpose(colv_bf[:, :], cs16[:, :])
            nc.gpsimd.tensor_copy(colv[:, :], colv_bf[:, :])

            # rb_sb[p, q] = csrow[q] broadcast down partitions (cnt_a source)
            for sgm in range(4):
                rbp = ps.tile([128, 1024], F32, tag="mega", bufs=2,
                              name=f"rbp_{sgm}")
                nc.tensor.matmul(rbp[:, 0:512], lhsT=ones_col[:, :],
                                 rhs=csrow_bf[:, 512 * sgm:512 * sgm + 512],
                                 start=True, stop=True)
                nc.vector.tensor_copy(rb_sb[:, 512 * sgm:512 * sgm + 512],
                                      rbp[:, 0:512])
            pcs.release()

        # ======== persistent QKV tiles ========
        qT = pm.tile([128, NH_LOC, S], BF16, tag="qT")
        kT = pm.tile([128, S], BF16, tag="kT")
        v_aug = pm.tile([128, NT, 132], BF16, tag="v_aug")
        nc.gpsimd.memset(v_aug[:, :, 128:129], 1.0)

        bounce_outs = {}
        afs_tiles = {}

        # ---------------- helpers ----------------
        def emit_ln1_pipe(g4, ln1T):
            # per-tile stats -> sqrt -> norm -> XBAR transpose chains so the
            # first transpose fires ~15us earlier than a batched-sqrt ladder
            for j4 in range(4):
                i = 4 * g4 + j4
                xt = xt_tiles.pop(i)
                st6 = pm.tile([128, 4, 6], F32, tag="st6a", bufs=2)
                for a4 in range(4):
                    nc.vector.bn_stats(st6[:, a4, :],
                                       xt[:, 512 * a4:512 * a4 + 512])
                nc.vector.bn_aggr(mv_all[:, i, :], st6[:, :, :])
                nc.scalar.activation(s_all[:, i:i + 1], mv_all[:, i, 1:2],
                                     AF.Sqrt, bias=eps_sb[:, :])
                nc.vector.reciprocal(rs_all[:, i:i + 1], s_all[:, i:i + 1])
                xh = pm.tile([128, DIM], BF16, tag="xh", bufs=2)
                nc.vector.tensor_scalar(
                    out=xh[:, :], in0=xt[:, :],
                    scalar1=mv_all[:, i, 0:1], scalar2=rs_all[:, i:i + 1],
                    op0=OP.subtract, op1=OP.mult)
                nc.sync.dma_start_transpose(
                    ln1T[:, :, 128 * j4:128 * j4 + 128], xh[:, :])

        def rope_sg(dst_sg, w_sb, h, sg, ln1T):
            bias_ap = kb_sb[:, 0:1] if h is None else qb_sb[:, h:h + 1]
            pq = ps.tile([128, 1024], F32, tag="mega", bufs=2,
                         name=f"pq_{h}_{sg}")
            for c in range(NC):
                lhsT = w_sb[:, c, :] if h is None \
                    else w_sb[:, c, 128 * h:128 * h + 128]
                nc.tensor.matmul(pq[:, 0:512], lhsT=lhsT,
                                 rhs=ln1T[:, c, 0:512],
                                 start=(c == 0), stop=(c == NC - 1))
            raw = pm.tile([128, 512], BF16, tag="rope_raw", bufs=2)
            nc.scalar.activation(raw[:, :], pq[:, 0:512], AF.Identity,
                                 bias=bias_ap)
            pw2 = ps.tile([128, 1024], F32, tag="mega", bufs=2,
                          name=f"pw2_{h}_{sg}")
            nc.tensor.matmul(pw2[:, 0:512], lhsT=pswap_sb[:, :],
                             rhs=raw[:, :], start=True, stop=True)
            t1 = pm.tile([128, 512], BF16, tag="rope_t1", bufs=2)
            nc.vector.tensor_mul(t1[:, :], raw[:, :],
                                 cos_sb[:, 512 * sg:512 * sg + 512])
            t2 = pm.tile([128, 512], BF16, tag="rope_t2", bufs=2)
            nc.vector.tensor_mul(t2[:, :], pw2[:, 0:512],
                                 sin_sb[:, 512 * sg:512 * sg + 512])
            nc.vector.tensor_add(dst_sg, t1[:, :], t2[:, :])

        def emit_qkv(g4, ln1T):
            sg = g4
            # K projection + rope for this seq block
            rope_sg(kT[:, 512 * sg:512 * sg + 512], wk_sb, None, sg, ln1T)
            # V projection for the 4 seq tiles of this block
            vps = ps.tile([128, 1024], F32, tag="mega", bufs=2,
                          name=f"vps_{g4}")
            for j4 in range(4):
                i = 4 * g4 + j4
                reg = vps[:, 128 * j4:128 * j4 + 128]
                for c in range(NC):
                    nc.tensor.matmul(
                        reg, lhsT=ln1T[:, c, 128 * j4:128 * j4 + 128],
                        rhs=wv_sb[:, c, :],
                        start=(c == 0), stop=(c == NC - 1),
                        skip_group_check=True)
            for j4 in range(4):
                i = 4 * g4 + j4
                nc.vector.tensor_add(v_aug[:, i, 0:128],
                                     vps[:, 128 * j4:128 * j4 + 128],
                                     vb_sb[:, :])
            # Q projections + rope
            for h in range(NH_LOC):
                rope_sg(qT[:, h, 512 * sg:512 * sg + 512], wq_sb, h, sg, ln1T)

        def build_mg(ci):
            s0, w = CHUNKS[ci]
            kts = list(range(max(0, s0 - 1), s0 + w))
            mg = pm.tile([128, 5, 512], BF16, tag="mg", bufs=1,
                         name=f"mg_{ci}")
            nc.vector.memset(mg[:, :, :], 0.0)
            mg_idx = {}
            for slot, kt in enumerate(kts):
                mg_idx[kt] = slot
                if s0 <= kt < s0 + w:  # diag: qtile t == kt
                    j = kt - s0
                    sl = mg[:, slot, 128 * j:128 * j + 128]
                    nc.vector.tensor_scalar(
                        out=sl, in0=rb_sb[:, 128 * kt:128 * kt + 128],
                        scalar1=colv[:, kt:kt + 1], scalar2=BIAS_SCALED,
                        op0=OP.is_lt, op1=OP.mult,
                    )
                    nc.vector.tensor_add(sl, sl, ctri_sb[:, :])
                tprev = kt + 1
                if s0 <= tprev < s0 + w:  # prev: qtile t == kt + 1
                    j = tprev - s0
                    sl = mg[:, slot, 128 * j:128 * j + 128]
                    nc.vector.tensor_scalar(
                        out=sl, in0=rb_sb[:, 128 * tprev:128 * tprev + 128],
                        scalar1=colv[:, kt:kt + 1], scalar2=BIAS_SCALED,
                        op0=OP.is_lt, op1=OP.mult,
                    )
            return mg, mg_idx

        def emit_attention(ci, mids=None, post_attn=None):
            s0, w = CHUNKS[ci]
            bounce_outs[ci] = dr.tile([4, 128 * w, DQ_LOC], BF16,
                                      name=f"bout{ci}", tag=f"bout{ci}")
            mg, mg_idx = build_mg(ci)
            attn_c = pm.tile([128, w, DQ_LOC], BF16, tag="attn_c",
                             bufs=2, name=f"attn_c{ci}")
            for h in range(NH_LOC):
                if mids is not None and h in mids:
                    mids[h]()
                nkt = s0 + w
                pvp = [ps.tile([128, 132], F32, tag="pv_acc", bufs=4,
                               name=f"pv_{h}_{ci}_{jj}")
                       for jj in range(w)]
                kt = 0
                while kt < nkt:
                    mega = ps.tile([128, 1024], F32, tag="mega", bufs=2,
                                   name=f"sc_{h}_{ci}_{kt}")
                    pair = [k2 for k2 in (kt, kt + 1) if k2 < nkt]
                    offs = []
                    for slot, k2 in enumerate(pair):
                        off = 128 * max(0, k2 - s0)
                        offs.append(off)
                        reg = mega[:, 512 * slot + off:
                                   512 * slot + 128 * w]
                        if k2 in mg_idx:
                            nc.tensor.matmul(
                                reg, lhsT=ident_sb[:, :],
                                rhs=mg[:, mg_idx[k2], off:128 * w],
                                start=True, stop=False)
                            nc.tensor.matmul(
                                reg,
                                lhsT=kT[:, 128 * k2:128 * k2 + 128],
                                rhs=qT[:, h, 128 * s0 + off:
                                       128 * (s0 + w)],
                                start=False, stop=True)
                        else:
                            nc.tensor.matmul(
                                reg,
                                lhsT=kT[:, 128 * k2:128 * k2 + 128],
                                rhs=qT[:, h, 128 * s0 + off:
                                       128 * (s0 + w)],
                                start=True, stop=True)
                    pt = pm.tile([128, 1024], BF16, tag="pt_sm", bufs=2)
                    if len(pair) == 2 and offs[0] == 0 and offs[1] == 0:
                        if w == 4:
                            nc.scalar.activation(pt[:, :], mega[:, :],
                                                 AF.Exp, scale=INV_SQD)
                        else:
                            for slot in range(2):
                                nc.scalar.activation(
                                    pt[:, 512 * slot:512 * slot + 128 * w],
                                    mega[:, 512 * slot:512 * slot + 128 * w],
                                    AF.Exp, scale=INV_SQD)
                    else:
                        for slot, k2 in enumerate(pair):
                            off = offs[slot]
                            nc.scalar.activation(
                                pt[:, 512 * slot + off:512 * slot + 128 * w],
                                mega[:, 512 * slot + off:512 * slot + 128 * w],
                                AF.Exp, scale=INV_SQD)
                    for slot, k2 in enumerate(pair):
                        for j in range(max(0, k2 - s0), w):
                            nc.tensor.matmul(
                                pvp[j][:, 0:129],
                                lhsT=pt[:, 512 * slot + 128 * j:
                                        512 * slot + 128 * j + 128],
                                rhs=v_aug[:, k2, 0:129],
                                start=(k2 == 0), stop=(k2 == s0 + j),
                                skip_group_check=True)
                    kt += len(pair)
                for j in range(w):
                    rcp = pm.tile([128, 1], F32, tag="rcp", bufs=4,
                                  name=f"rcp_{h}_{ci}_{j}")
                    nc.vector.reciprocal(rcp[:, :], pvp[j][:, 128:129])
                    nc.vector.tensor_single_scalar(
                        out=attn_c[:, j, 128 * h:128 * h + 128],
                        in_=pvp[j][:, 0:128],
                        scalar=rcp[:, :],
                        op=OP.mult)

            bounce_in = dr.tile([128 * w, DQ_LOC], BF16, name=f"bin{ci}",
                                tag=f"bin{ci}")
            bounce_out = bounce_outs[ci]
            nc.sync.dma_start(
                out=bounce_in.rearrange("(t p) n -> p t n", p=128),
                in_=attn_c[:, :, :])
            nc.gpsimd.collective_compute(
                "AllGather",
                mybir.AluOpType.bypass,
                replica_groups=[[0, 1, 2, 3], [4, 5, 6, 7]],
                ins=[bounce_in[:, :].opt()],
                outs=[bounce_out[:, :, :].opt()],
            )
            if post_attn is not None:
                post_attn()

        def emit_ln2_stats(ci):
            s0, w = CHUNKS[ci]
            for j in range(w):
                t = s0 + j
                afs = pm.tile([128, 4, 512], BF16, tag="afs", bufs=4,
                              name=f"afs_{t}")
                nc.sync.dma_start(
                    out=afs[:, :, :],
                    in_=bounce_outs[ci][:, 128 * j:128 * j + 128, :]
                        .rearrange("g p n -> p g n"))
                afs_tiles[t] = afs
                st6b = pm.tile([128, 4, 6], F32, tag="st6b", bufs=2)
                for a4 in range(4):
                    nc.vector.bn_stats(st6b[:, a4, :], afs[:, a4, :])
                nc.vector.bn_aggr(mv2_all[:, t, :], st6b[:, :, :])

        def emit_ln2_sqrt(ci):
            s0, w = CHUNKS[ci]
            sl = slice(s0, s0 + w)
            nc.scalar.activation(s2_all[:, sl], mv2_all[:, sl, 1:2], AF.Sqrt,
                                 bias=eps_sb[:, :])
            nc.vector.reciprocal(rs2_all[:, sl], s2_all[:, sl])

        def emit_ln2_mm(ci):
            s0, w = CHUNKS[ci]
            for j in range(w):
                t = s0 + j
                afs = afs_tiles.pop(t)
                xh2 = pm.tile([128, DIM], BF16, tag="xh2", bufs=2,
                              name=f"xh2_{t}")
                nc.vector.tensor_scalar(
                    out=xh2[:, :],
                    in0=afs.rearrange("p g n -> p (g n)"),
                    scalar1=mv2_all[:, t, 0:1], scalar2=rs2_all[:, t:t + 1],
                    op0=OP.subtract, op1=OP.mult)
                ln2T = pm.tile([128, NC, 128], BF16, tag="ln2T", bufs=2,
                               name=f"ln2T_{t}")
                nc.sync.dma_start_transpose(ln2T[:, :, :], xh2[:, :])
                po = ps.tile([128, 1024], F32, tag="mega", bufs=2,
                             name=f"po_{t}")
                for c in range(NC):
                    nc.tensor.matmul(
                        po[:, 0:512],
                        lhsT=ln2T[:, c, :],
                        rhs=wo_sb[:, c, :],
                        start=(c == 0), stop=(c == NC - 1))
                osb = pm.tile([128, DQ_LOC], F32, tag="osb", bufs=2,
                              name=f"osb_{t}")
                nc.vector.tensor_add(osb[:, :], po[:, 0:512], ob_sb[:, :])
                nc.sync.dma_start(out=out[128 * t:128 * t + 128, :],
                                   in_=osb[:, :])

        # ================= pipelined columns =================
        # col g4: [prefetch x(g4+1)] LN1(g4) QKV(g4) [ln2 stats]
        #         ATTN(chunk) [+AG] [ln2 mm]
        def emit_ln1_block(g4):
            # full LN1 chain for column g4; DMA lands before any later
            # collective parks the queues.
            if g4 < 4:
                ln1T = pm.tile([128, NC, 512], BF16, tag="ln1T", bufs=2,
                               name=f"ln1T_{g4}")
                emit_ln1_pipe(g4, ln1T)
                ln1T_tiles[g4] = ln1T

        def emit_ln2_block(ci):
            emit_ln2_stats(ci)
            emit_ln2_sqrt(ci)
            emit_ln2_mm(ci)

        def emit_ln2_tail(ci):
            # per-tile pipelined chain; emitted BEFORE the chunk's bounce/AG
            # so every DMA is queued ahead of the collective's descriptors
            s0, w = CHUNKS[ci]
            for j in range(w):
                t = s0 + j
                afs = pm.tile([128, 4, 512], BF16, tag="afs", bufs=4,
                              name=f"afs_{t}")
                nc.sync.dma_start(
                    out=afs[:, :, :],
                    in_=bounce_outs[ci][:, 128 * j:128 * j + 128, :]
                        .rearrange("g p n -> p g n"))
                st6b = pm.tile([128, 4, 6], F32, tag="st6b", bufs=2)
                for a4 in range(4):
                    nc.vector.bn_stats(st6b[:, a4, :], afs[:, a4, :])
                nc.vector.bn_aggr(mv2_all[:, t, :], st6b[:, :, :])
                nc.scalar.activation(s2_all[:, t:t + 1], mv2_all[:, t, 1:2],
                                     AF.Sqrt, bias=eps_sb[:, :])
                nc.vector.reciprocal(rs2_all[:, t:t + 1], s2_all[:, t:t + 1])
                xh2 = pm.tile([128, DIM], BF16, tag="xh2", bufs=2,
                              name=f"xh2_{t}")
                nc.vector.tensor_scalar(
                    out=xh2[:, :],
                    in0=afs.rearrange("p g n -> p (g n)"),
                    scalar1=mv2_all[:, t, 0:1], scalar2=rs2_all[:, t:t + 1],
                    op0=OP.subtract, op1=OP.mult)
                ln2T = pm.tile([128, NC, 128], BF16, tag="ln2T", bufs=2,
                               name=f"ln2T_{t}")
                nc.sync.dma_start_transpose(ln2T[:, :, :], xh2[:, :])
                po = ps.tile([128, 1024], F32, tag="mega", bufs=2,
                             name=f"po_{t}")
                for c in range(NC):
                    nc.tensor.matmul(
                        po[:, 0:512],
                        lhsT=ln2T[:, c, :],
                        rhs=wo_sb[:, c, :],
                        start=(c == 0), stop=(c == NC - 1))
                osb = pm.tile([128, DQ_LOC], F32, tag="osb", bufs=2,
                              name=f"osb_{t}")
                nc.vector.tensor_add(osb[:, :], po[:, 0:512], ob_sb[:, :])
                nc.sync.dma_start(out=out[128 * t:128 * t + 128, :],
                                  in_=osb[:, :])

        ln1T_tiles = {}
        nc.scalar.dma_start(out=lab_sb[:, :],
                            in_=labels.ap().rearrange("(o s) -> o s", o=1))
        emit_early_posts()
        emit_ln1_block(0)
        emit_late_posts()
        for g4 in range(4):
            if g4 < 3:
                for i in range(4 * g4 + 4, 4 * g4 + 8):
                    load_xt(i)
            emit_qkv(g4, ln1T_tiles.pop(g4))
            if g4 == 0:
                emit_csum()
            mids = {}
            if g4 < 3:
                mids[1] = (lambda n=g4 + 1: emit_ln1_block(n))
            if g4 == 2:
                mids[2] = (lambda: emit_ln2_block(0))
            post = None
            if g4 == 3:
                mids[2] = (lambda: emit_ln2_block(1))
                mids[3] = (lambda: emit_ln2_block(2))
                post = (lambda: emit_ln2_tail(3))
            emit_attention(g4, mids=mids, post_attn=post)

    nc.compile()
    return nc


def _prep_inputs(x, freqs_cis, seizure_labels, wq, wk, wv, wo,
                 ln1_w, ln1_b, ln2_w, ln2_b):
    bf16 = ml_dtypes.bfloat16
    cos = np.asarray(freqs_cis[..., 0], dtype=np.float32)  # [S, 64]
    sin = np.asarray(freqs_cis[..., 1], dtype=np.float32)
    cosT = np.ascontiguousarray(np.repeat(cos.T, 2, axis=0), dtype=bf16)
    sgn = np.where(np.arange(HEAD_DIM) % 2 == 0, -1.0, 1.0).astype(np.float32)
    sinT = np.ascontiguousarray(np.repeat(sin.T, 2, axis=0) * sgn[:, None],
                                dtype=bf16)
    ident = np.eye(128, dtype=bf16)
    psw = np.zeros((128, 128), dtype=np.float32)
    idx = np.arange(128)
    psw[idx ^ 1, idx] = 1.0  # out[m, s] = sum_k psw[k, m] * in[k, s] = in[m^1, s]
    psw = psw.astype(bf16)
    kk = np.arange(128)[:, None]
    qq = np.arange(128)[None, :]
    ctri_np = np.where(qq >= kk, 0.0, NEG_SCALED).astype(bf16)

    # fold LN affine weights into the projection weights (host-side
    # preprocessing, standard inference-time weight folding):
    #   ln(x)@W.T = xhat@(W*w).T + b@W.T
    w1 = np.asarray(ln1_w, np.float64)
    b1 = np.asarray(ln1_b, np.float64)
    w2 = np.asarray(ln2_w, np.float64)
    b2 = np.asarray(ln2_b, np.float64)
    in_maps = []
    for cid in range(8):
        b, g = divmod(cid, 4)
        wq_s = np.asarray(wq[DQ_LOC * g:DQ_LOC * (g + 1), :], np.float64)
        wk_s = np.asarray(wk[HEAD_DIM * g:HEAD_DIM * (g + 1), :], np.float64)
        wv_s = np.asarray(wv[HEAD_DIM * g:HEAD_DIM * (g + 1), :], np.float64)
        wo_s = np.asarray(wo[DQ_LOC * g:DQ_LOC * (g + 1), :], np.float64)
        qb_v = (b1 @ wq_s.T).astype(np.float32)         # [512]
        kb_v = (b1 @ wk_s.T).astype(np.float32)         # [128]
        vb_v = (b1 @ wv_s.T).astype(np.float32)         # [128]
        ob_v = (b2 @ wo_s.T).astype(np.float32)         # [512]
        in_maps.append({
            "xs": np.ascontiguousarray(x[b], dtype=bf16),
            "wqT": np.ascontiguousarray((wq_s * w1).T, dtype=bf16),
            "wkT": np.ascontiguousarray((wk_s * w1).T, dtype=bf16),
            "wvT": np.ascontiguousarray((wv_s * w1).T, dtype=bf16),
            "woT": np.ascontiguousarray((wo_s * w2).T, dtype=bf16),
            "qb": np.ascontiguousarray(
                qb_v.reshape(NH_LOC, 128).T, dtype=np.float32),
            "kb": np.ascontiguousarray(kb_v.reshape(128, 1), dtype=np.float32),
            "vbt": np.ascontiguousarray(np.tile(vb_v, (128, 1)), dtype=bf16),
            "obt": np.ascontiguousarray(np.tile(ob_v, (128, 1)),
                                        dtype=np.float32),
            "labels": np.ascontiguousarray(seizure_labels[b], dtype=bf16),
            "cosT": cosT, "sinT": sinT,
            "ident": ident, "pswap": psw, "ctri": ctri_np,
        })
    return in_maps


def run(inputs, trace=False, trace_cores=None):
    x = np.asarray(inputs["x"])
    mask = np.asarray(inputs["mask"])
    # this kernel specializes the additive mask to the causal prefill mask
    causal = np.where(np.tril(np.ones((S, S), dtype=bool)), 0.0, NEG_INF
                      ).astype(np.float32)
    if not np.array_equal(mask, causal):
        raise NotImplementedError("kernel specialized for causal prefill mask")

    in_maps = _prep_inputs(
        x, np.asarray(inputs["freqs_cis"]), np.asarray(inputs["seizure_labels"]),
        np.asarray(inputs["wq"]), np.asarray(inputs["wk"]),
        np.asarray(inputs["wv"]), np.asarray(inputs["wo"]),
        np.asarray(inputs["ln1_w"]), np.asarray(inputs["ln1_b"]),
        np.asarray(inputs["ln2_w"]), np.asarray(inputs["ln2_b"]))

    if "nc" not in _CACHED:
        _CACHED["nc"] = build_nc()
    nc = _CACHED["nc"]

    kw = {}
    if trace:
        kw = dict(trace=True,
                  trace_cores=trace_cores if trace_cores is not None else [0])
    res = run_bass_kernel_spmd(nc, in_maps, core_ids=list(range(8)), **kw)

    shards = [res.results[cid]["out"] for cid in range(8)]
    full = np.empty((B, S, DIM), dtype=np.float32)
    for cid in range(8):
        b, g = divmod(cid, 4)
        full[b, :, DQ_LOC * g:DQ_LOC * (g + 1)] = shards[cid]
    return full, res


def kernel(**inputs) -> np.ndarray:
    out, _ = run(inputs, trace=False)
    return out

